# revision 11
# baseline (speedup 1.0000x reference)
"""Trainium2 Bass kernel for nn_AutoRegressiveGraphConvLayer.

Self-contained: host-side layout prep (padding, transposes, bf16 cast, weight
stacking, mask/norm tables) + an 8-core SPMD Bass kernel + output reassembly.

Sharding: 8 cores = 4 batch samples x 2 node-range halves (nodes 0..511 /
512..1023). Each core handles T = 512*32 = 16384 padded edge slots (slot =
32*li + r; pad-at-start so the block structure is uniform) and 512 nodes.

Device pipeline (feature-major, bf16 operands, fp32 PSUM):
  X = [E(0:32) | padmask(32) | node_j(33:97)]   (one DMA; node_j im2col'd on host)
  t1 = relu(W_t1a.T @ X[0:97] + W_t1b.T @ node_i_bcast)        [128, T]
  t2 = relu(W_t2.T @ t1 + ban1)                 group-stacked   [128, T/2]
  prev_n = windowed reduce_sum(t2*mask) * norm  -> node MLP -> out_n
  s1 = relu(W_s1.T @ X[0:97])                   group-stacked   [128, T/2]
  s2 = relu(W_s2.T @ s1 + bae1)                 group-stacked   [128, T/4]
  prev_e = (segmented_scan(s2) - s2) * nrm      -> X[33:65]
  gl0 = relu(W_gl0.T @ X[0:65])                 group-stacked   [128, T/2]
  out_e = relu(W_oute.T @ gl0 + ble1)           group-stacked   [128, T/4]
All inputs arrive as 3 DRAM tensors per core (xinit, wblob bf16, fblob f32)
to keep per-instruction semaphore fan-in low.
"""

import os

import numpy as np
import ml_dtypes

import concourse.bass as bass
import concourse.tile as tile
from concourse import bacc, mybir
from concourse.bass_utils import run_bass_kernel_spmd

BF = mybir.dt.bfloat16
F32 = mybir.dt.float32

N, M, B = 1024, 32, 4
NL = 512                 # local nodes per core
T = NL * 32              # padded edge slots per core
NCH = 512                # matmul moving chunk
NCHUNKS = T // NCH       # 32
TG2, TG4 = T // 2, T // 4

LAST_EXEC_NS = None

# weight-blob column layout: name -> (row_count, col_offset, col_count)
_WSEG = {}
_off = 0
for _name, _rows, _cols in [
    ("w_t1a", 97, 128), ("w_t1b", 64, 128), ("w_s1", 97, 64),
    ("w_t2", 128, 64), ("w_s2d", 128, 32), ("w_gl0", 65, 64),
    ("w_outed", 128, 32), ("w_hl0a", 64, 128), ("w_hl0b", 64, 128),
    ("w_nl1", 128, 64), ("nodesT", 64, 544),
    ("scanmask", 128, TG4), ("nrm_gs", 128, TG4), ("normn", 128, 256),
    ("t2mask", 64, 1024), ("s2mask", 32, 1024),
]:
    _WSEG[_name] = (_rows, _off, _cols)
    _off += _cols
WBLOB_COLS = _off

_FSEG = {"ban1_t": (128, 0), "bae1_t": (128, 1), "ble1_t": (128, 2),
         "bln0_t": (128, 3), "bln1_t": (64, 4)}
FBLOB_COLS = 5


def _bf(x):
    return np.ascontiguousarray(np.asarray(x).astype(ml_dtypes.bfloat16))


def _f32(x):
    return np.ascontiguousarray(np.asarray(x).astype(np.float32))


# --------------------------------------------------------------------------
# static index tables (mirrors reference._build_indices structure)
# --------------------------------------------------------------------------
_t_i = np.minimum(np.arange(N), M)
_S = np.concatenate([[0], np.cumsum(_t_i)]).astype(np.int64)   # block starts
N_E = int(_S[N])                                               # 32240


def _core_tables(i0):
    li = np.arange(NL)
    gi = i0 + li
    tt = np.minimum(gi, 32)
    r = np.arange(32)
    real = r[None, :] >= (32 - tt[:, None])                    # [NL, 32]
    eidx = _S[gi][:, None] + (r[None, :] - (32 - tt[:, None]))  # [NL, 32]
    return real, eidx, tt


def _host_prep(b, h, inputs):
    """Build the per-core input map: xinit [97,T], wblob [128,WC], fblob."""
    i0 = 512 * h
    real, eidx, tt = _core_tables(i0)
    r = np.arange(32)
    li = np.arange(NL)

    nodes = _f32(inputs["input_nodes"][b])                     # [1024, 64]
    edges = _f32(inputs["input_edges"][b])                     # [N_E, 32]

    # xinit = [edgesT(0:32) | padmask(32) | node_j(33:97)]
    xinit = np.zeros((97, T), np.float32)
    E_pad = np.zeros((NL, 32, 32), np.float32)
    E_pad[real] = edges[eidx[real]]
    xinit[0:32] = E_pad.reshape(T, 32).T
    xinit[32] = real.reshape(T)

    halo = np.zeros((544, 64), np.float32)
    jj = np.arange(i0 - 32, i0 + 512)
    halo[jj >= 0] = nodes[jj[jj >= 0]]
    nj = (li[:, None] + r[None, :]).reshape(T)
    xinit[33:97] = halo.T[:, nj]

    # weights
    Wan0, ban0 = inputs["Wan0"], inputs["ban0"]
    Wan1, ban1 = inputs["Wan1"], inputs["ban1"]
    Wln0, bln0 = inputs["Wln0"], inputs["bln0"]
    Wln1, bln1 = inputs["Wln1"], inputs["bln1"]
    Wae0, bae0 = inputs["Wae0"], inputs["bae0"]
    Wae1, bae1 = inputs["Wae1"], inputs["bae1"]
    Wle0, ble0 = inputs["Wle0"], inputs["ble0"]
    Wle1, ble1 = inputs["Wle1"], inputs["ble1"]

    tok = np.arange(T)
    m = (tok % 32 != 0).astype(np.float32)
    cnt = (r[None, :] - (32 - tt[:, None])).reshape(T).astype(np.float32)
    nrm = np.where(cnt > 0, 1.0 / np.maximum(cnt, 1), 1.0).astype(np.float32)
    norm_node = np.full(N, 1.0 / M, np.float32)
    norm_node[1:M] = 1.0 / np.arange(1, M)
    nv = norm_node[i0 + np.arange(NL)]

    seg = {
        "w_t1a": np.concatenate([np.asarray(Wan0)[64:96], np.asarray(ban0)[None, :],
                                 np.asarray(Wan0)[0:64]], 0),
        "w_t1b": np.asarray(Wan0)[96:160],
        "w_s1": np.concatenate([np.asarray(Wae0)[64:96], np.asarray(bae0)[None, :],
                                np.asarray(Wae0)[0:64]], 0),
        "w_t2": np.asarray(Wan1),
        "w_s2d": np.concatenate([np.asarray(Wae1), np.asarray(Wae1)], 0),
        "w_gl0": np.concatenate([np.asarray(Wle0)[32:64], np.asarray(ble0)[None, :],
                                 np.asarray(Wle0)[0:32]], 0),
        "w_outed": np.concatenate([np.asarray(Wle1), np.asarray(Wle1)], 0),
        "w_hl0a": np.asarray(Wln0)[0:64],
        "w_hl0b": np.asarray(Wln0)[64:128],
        "w_nl1": np.asarray(Wln1),
        "nodesT": halo.T,
        "scanmask": np.broadcast_to(m.reshape(4, TG4)[:, None, :],
                                    (4, 32, TG4)).reshape(128, TG4),
        "nrm_gs": np.broadcast_to(nrm.reshape(4, TG4)[:, None, :],
                                  (4, 32, TG4)).reshape(128, TG4),
        "normn": np.broadcast_to(nv.reshape(2, 256)[:, None, :],
                                 (2, 64, 256)).reshape(128, 256),
        "t2mask": real.reshape(T)[:1024][None, :].repeat(64, 0),
        "s2mask": real.reshape(T)[:1024][None, :].repeat(32, 0),
    }
    wblob = np.zeros((128, WBLOB_COLS), np.float32)
    for name, arr in seg.items():
        rows, off, cols = _WSEG[name]
        assert arr.shape == (rows, cols), (name, arr.shape)
        wblob[0:rows, off:off + cols] = arr

    fblob = np.zeros((128, FBLOB_COLS), np.float32)
    fblob[0:128, 0] = np.concatenate([ban1, ban1])
    fblob[0:128, 1] = np.tile(np.asarray(bae1), 4)
    fblob[0:128, 2] = np.tile(np.asarray(ble1), 4)
    fblob[0:128, 3] = np.asarray(bln0)
    fblob[0:64, 4] = np.asarray(bln1)

    return {"xinit": _bf(xinit), "wblob": _bf(wblob), "fblob": _f32(fblob)}


# --------------------------------------------------------------------------
# device kernel
# --------------------------------------------------------------------------
def build_kernel(debug=False):
    nc = bacc.Bacc("TRN2", target_bir_lowering=False, debug=debug)
    AF = mybir.ActivationFunctionType
    ALU = mybir.AluOpType

    xinit_d = nc.dram_tensor("xinit", [97, T], BF, kind="ExternalInput").ap()
    wblob_d = nc.dram_tensor("wblob", [128, WBLOB_COLS], BF, kind="ExternalInput").ap()
    fblob_d = nc.dram_tensor("fblob", [128, FBLOB_COLS], F32, kind="ExternalInput").ap()
    out_n = nc.dram_tensor("out_n", [64, NL], F32, kind="ExternalOutput").ap()
    out_e = nc.dram_tensor("out_e", [128, TG4], BF, kind="ExternalOutput").ap()

    with tile.TileContext(nc) as tc:
        with (
            tc.tile_pool(name="big", bufs=1) as big,
            tc.tile_pool(name="pt1", bufs=2, space="PSUM") as pt1p,
            tc.tile_pool(name="ppk", bufs=4, space="PSUM") as ppk,
            tc.tile_pool(name="pnd", bufs=2, space="PSUM") as pnd,
        ):
            X = big.tile([128, T], BF, tag="X")
            t1 = big.tile([128, T], BF, tag="t1")
            S = big.tile([128, TG2], BF, tag="S")
            t2 = big.tile([128, TG2], BF, tag="t2")
            s2m = big.tile([128, TG4], BF, tag="s2m")
            incl = big.tile([128, TG4], BF, tag="incl")
            excl = big.tile([128, TG4], BF, tag="excl")
            Gg = big.tile([128, TG2], BF, tag="G")
            oute = big.tile([128, TG4], BF, tag="oute")
            W = big.tile([128, WBLOB_COLS], BF, tag="W")
            Fb = big.tile([128, FBLOB_COLS], F32, tag="Fb")
            prevn_raw = big.tile([128, 256], F32, tag="prevn_raw")
            prevn = big.tile([128, 256], BF, tag="prevn")
            pn2 = big.tile([64, 512], BF, tag="pn2")
            hl0 = big.tile([128, 512], BF, tag="hl0")
            outn_sb = big.tile([64, 512], F32, tag="outn_sb")

            def w(name):
                rows, off, cols = _WSEG[name]
                return W[0:rows, off:off + cols]

            def f(name):
                rows, col = _FSEG[name]
                return Fb[0:rows, col:col + 1]

            dma = nc.sync.dma_start
            dma(out=X[0:97, :], in_=xinit_d)
            dma(out=W[:, :], in_=wblob_d)
            dma(out=Fb[:, :], in_=fblob_d)

            def cw(c):
                return slice(c * NCH, (c + 1) * NCH)

            def node_i_bcast(c):
                # rhs [64, 16 blocks, 32 bcast] reading nodesT cols li+32
                base = w("nodesT")[:, 32 + 16 * c: 32 + 16 * c + 16]
                return bass.AP(tensor=base.tensor, offset=base.offset,
                               ap=[base.ap[0], base.ap[1], [0, 32]])

            # ---- t1 ------------------------------------------------------
            for c in range(NCHUNKS):
                pt = pt1p.tile([128, NCH], F32, tag="pt1")
                nc.tensor.matmul(pt[:, :], w("w_t1a"), X[0:97, cw(c)],
                                 start=True, stop=False)
                nc.tensor.matmul(pt[:, :], w("w_t1b"), node_i_bcast(c),
                                 start=False, stop=True)
                if c % 2 == 0:
                    nc.scalar.activation(t1[:, cw(c)], pt[:, :], AF.Relu)
                else:
                    nc.vector.tensor_scalar_max(t1[:, cw(c)], pt[:, :], 0.0)

            # ---- s1 (pack 2 chunks/psum tile) ---------------------------
            for cp in range(16):
                ps = ppk.tile([128, NCH], F32, tag="ppk")
                nc.tensor.matmul(ps[0:64, :], w("w_s1"), X[0:97, cw(cp)],
                                 start=True, stop=True)
                nc.tensor.matmul(ps[64:128, :], w("w_s1"), X[0:97, cw(cp + 16)],
                                 start=True, stop=True)
                if cp % 2 == 0:
                    nc.scalar.activation(S[:, cw(cp)], ps[:, :], AF.Relu)
                else:
                    nc.vector.tensor_scalar_max(S[:, cw(cp)], ps[:, :], 0.0)

            # ---- t2 (pack 2, ACT bias ban1) -----------------------------
            for cp in range(16):
                ps = ppk.tile([128, NCH], F32, tag="ppk")
                nc.tensor.matmul(ps[0:64, :], w("w_t2"), t1[:, cw(cp)],
                                 start=True, stop=True)
                nc.tensor.matmul(ps[64:128, :], w("w_t2"), t1[:, cw(cp + 16)],
                                 start=True, stop=True)
                nc.scalar.activation(t2[:, cw(cp)], ps[:, :], AF.Relu,
                                     bias=f("ban1_t"))

            # ---- node aggregation ---------------------------------------
            nc.vector.tensor_mul(t2[0:64, 0:1024], t2[0:64, 0:1024],
                                 w("t2mask"))
            t2v = t2[:, :].rearrange("p (b r) -> p b r", r=32)
            nc.vector.reduce_sum(out=prevn_raw[:, :], in_=t2v,
                                 axis=mybir.AxisListType.X)
            nc.vector.tensor_mul(prevn[:, :], prevn_raw[:, :], w("normn"))
            dma(out=pn2[0:64, 0:256], in_=prevn[0:64, :])
            dma(out=pn2[0:64, 256:512], in_=prevn[64:128, :])

            # ---- node MLP ------------------------------------------------
            ph = pnd.tile([128, 512], F32, tag="pnd")
            nc.tensor.matmul(ph[:, :], w("w_hl0a"), pn2[:, :],
                             start=True, stop=False)
            nc.tensor.matmul(ph[:, :], w("w_hl0b"), w("nodesT")[:, 32:544],
                             start=False, stop=True)
            nc.scalar.activation(hl0[:, :], ph[:, :], AF.Relu, bias=f("bln0_t"))
            po = pnd.tile([128, 512], F32, tag="pnd")
            nc.tensor.matmul(po[0:64, :], w("w_nl1"), hl0[:, :],
                             start=True, stop=True)
            nc.scalar.activation(outn_sb[:, :], po[0:64, :], AF.Relu,
                                 bias=f("bln1_t"))
            dma(out=out_n, in_=outn_sb[:, :])

            # ---- s2 (pack 4, ACT bias bae1) -----------------------------
            for cq in range(8):
                ps = ppk.tile([128, NCH], F32, tag="ppk")
                for k in range(4):
                    c = cq + 8 * k
                    g = c // 16
                    col = slice(c * NCH - g * TG2, (c + 1) * NCH - g * TG2)
                    nc.tensor.matmul(ps[32 * k:32 * k + 32, :],
                                     w("w_s2d")[64 * g:64 * g + 64, :],
                                     S[64 * g:64 * g + 64, col],
                                     start=True, stop=True,
                                     tile_position=(64 * g, 32 * k))
                nc.scalar.activation(s2m[:, cw(cq)], ps[:, :], AF.Relu,
                                     bias=f("bae1_t"))
            nc.vector.tensor_mul(s2m[0:32, 0:1024], s2m[0:32, 0:1024],
                                 w("s2mask"))

            # ---- edge prefix scan ---------------------------------------
            nc.vector.tensor_tensor_scan(incl[:, :], w("scanmask"), s2m[:, :],
                                         0.0, ALU.mult, ALU.add)
            nc.vector.tensor_sub(excl[:, :], incl[:, :], s2m[:, :])
            nc.vector.tensor_mul(excl[:, :], excl[:, :], w("nrm_gs"))
            for g in range(4):
                dma(out=X[33:65, g * TG4:(g + 1) * TG4],
                    in_=excl[32 * g:32 * g + 32, :])

            # ---- gl0 (pack 2) -------------------------------------------
            for cp in range(16):
                ps = ppk.tile([128, NCH], F32, tag="ppk")
                nc.tensor.matmul(ps[0:64, :], w("w_gl0"), X[0:65, cw(cp)],
                                 start=True, stop=True)
                nc.tensor.matmul(ps[64:128, :], w("w_gl0"), X[0:65, cw(cp + 16)],
                                 start=True, stop=True)
                if cp % 2 == 0:
                    nc.scalar.activation(Gg[:, cw(cp)], ps[:, :], AF.Relu)
                else:
                    nc.vector.tensor_scalar_max(Gg[:, cw(cp)], ps[:, :], 0.0)

            # ---- out_e (pack 4, ACT bias ble1) --------------------------
            for cq in range(8):
                ps = ppk.tile([128, NCH], F32, tag="ppk")
                for k in range(4):
                    c = cq + 8 * k
                    g = c // 16
                    col = slice(c * NCH - g * TG2, (c + 1) * NCH - g * TG2)
                    nc.tensor.matmul(ps[32 * k:32 * k + 32, :],
                                     w("w_outed")[64 * g:64 * g + 64, :],
                                     Gg[64 * g:64 * g + 64, col],
                                     start=True, stop=True,
                                     tile_position=(64 * g, 32 * k))
                nc.scalar.activation(oute[:, cw(cq)], ps[:, :], AF.Relu,
                                     bias=f("ble1_t"))
            dma(out=out_e, in_=oute[:, :])

    if not nc.is_finalized():
        nc.finalize()
    return nc


def _install_ntff_shim():
    """Provide antenv.axon_hooks (missing on this image) so trace=True can
    capture NTFF profiles via the axon .so C ABI. Only used when KERNEL_TRACE
    is set; the plain kernel() path never imports it."""
    import contextlib
    import ctypes
    import sys as _sys
    import types

    try:
        from antenv.axon_hooks import get_axon_ntff_profile_hook  # noqa: F401
        return
    except ImportError:
        pass

    so_path = "/opt/axon/libaxon_pjrt.so"
    hook = None
    try:
        lib = ctypes.CDLL(so_path)
        if hasattr(lib, "axon_start_nrt_profile"):
            lib.axon_start_nrt_profile.argtypes = [
                ctypes.POINTER(ctypes.c_int64), ctypes.c_size_t]
            lib.axon_start_nrt_profile.restype = ctypes.c_int64
            lib.axon_stop_nrt_profile.argtypes = [ctypes.c_char_p]
            lib.axon_stop_nrt_profile.restype = ctypes.c_int64

            @contextlib.contextmanager
            def _hook(output_dir, device_ids):
                import jax
                jax.devices()
                if device_ids:
                    ids = (ctypes.c_int64 * len(device_ids))(*device_ids)
                    rc = lib.axon_start_nrt_profile(ids, len(device_ids))
                else:
                    rc = lib.axon_start_nrt_profile(None, 0)
                if rc != 0:
                    raise RuntimeError(f"axon_start_nrt_profile rc={rc}")
                try:
                    yield
                finally:
                    n = lib.axon_stop_nrt_profile(str(output_dir).encode())
                    print(f"ntff profile: {n} file(s) -> {output_dir}")

            hook = _hook
    except OSError:
        pass

    mod = types.ModuleType("antenv.axon_hooks")
    mod._hook = hook
    mod.get_axon_ntff_profile_hook = lambda: mod._hook
    mod.set_axon_ntff_profile_hook = lambda h: setattr(mod, "_hook", h)
    import antenv
    antenv.axon_hooks = mod
    _sys.modules["antenv.axon_hooks"] = mod


# --------------------------------------------------------------------------
# host entry point
# --------------------------------------------------------------------------
def kernel(**inputs):
    in_maps = []
    metas = []
    for core in range(8):
        b, h = core // 2, core % 2
        in_maps.append(_host_prep(b, h, inputs))
        metas.append((b, h))

    nc = build_kernel(debug=False)
    trace = bool(os.environ.get("KERNEL_TRACE"))
    if trace:
        _install_ntff_shim()
    res = run_bass_kernel_spmd(nc, in_maps, core_ids=list(range(8)), trace=trace)
    global LAST_EXEC_NS
    LAST_EXEC_NS = res.exec_time_ns
    results = res.results

    output_nodes = np.zeros((B, N, 64), np.float32)
    output_edges = np.zeros((B, N_E, 32), np.float32)
    for core, (b, h) in enumerate(metas):
        i0 = 512 * h
        real, eidx, _ = _core_tables(i0)
        on = np.asarray(results[core]["out_n"], np.float32)         # [64, 512]
        oe = np.asarray(results[core]["out_e"]).astype(np.float32)  # [128, TG4]
        output_nodes[b, i0:i0 + NL] = on.T
        # unstack: partition p = f + 32*k, col c -> slot = k*TG4 + c
        oe4 = oe.reshape(4, 32, TG4)
        slots = np.transpose(oe4, (0, 2, 1)).reshape(T, 32)
        blk = slots.reshape(NL, 32, 32)
        output_edges[b][eidx[real]] = blk[real]
    return output_nodes, output_edges


# revision 19
# speedup vs baseline: 2.0079x; 2.0079x over previous
"""Trainium2 Bass kernel for nn_AutoRegressiveGraphConvLayer.

Self-contained: host-side layout prep (padding, transposes, bf16 cast, weight
stacking, mask/norm tables) + an 8-core SPMD Bass kernel + output reassembly.

Sharding: 8 cores = 4 batch samples x 2 node-range halves (nodes 0..511 /
512..1023). Each core handles T = 512*32 = 16384 padded edge slots (slot =
32*li + r; pad-at-start so the block structure is uniform) and 512 nodes.

Device pipeline (feature-major, bf16 operands, fp32 PSUM):
  X = [E(0:32) | padmask(32) | node_j(33:97)]   (one DMA; node_j im2col'd on host)
  t1 = relu(W_t1a.T @ X[0:97] + W_t1b.T @ node_i_bcast)        [128, T]
  t2 = relu(W_t2.T @ t1 + ban1)                 group-stacked   [128, T/2]
  prev_n = windowed reduce_sum(t2*mask) * norm  -> node MLP -> out_n
  s1 = relu(W_s1.T @ X[0:97])                   group-stacked   [128, T/2]
  s2 = relu(W_s2.T @ s1 + bae1)                 group-stacked   [128, T/4]
  prev_e = (segmented_scan(s2) - s2) * nrm      -> X[33:65]
  gl0 = relu(W_gl0.T @ X[0:65])                 group-stacked   [128, T/2]
  out_e = relu(W_oute.T @ gl0 + ble1)           group-stacked   [128, T/4]
All inputs arrive as 3 DRAM tensors per core (xinit, wblob bf16, fblob f32)
to keep per-instruction semaphore fan-in low.
"""

import os

import numpy as np
import ml_dtypes

import concourse.bass as bass
import concourse.tile as tile
from concourse import bacc, mybir
from concourse.bass_utils import run_bass_kernel_spmd

BF = mybir.dt.bfloat16
F32 = mybir.dt.float32

N, M, B = 1024, 32, 4
NL = 512                 # local nodes per core
T = NL * 32              # padded edge slots per core
NCH = 512                # matmul moving chunk
NCHUNKS = T // NCH       # 32
TG2, TG4 = T // 2, T // 4

LAST_EXEC_NS = None

# weight-blob column layout: name -> (row_count, col_offset, col_count)
_WSEG = {}
_off = 0
for _name, _rows, _cols in [
    ("w_t1a", 97, 128), ("w_t1b", 64, 128), ("w_s1", 97, 64),
    ("w_t2", 128, 64), ("w_s2d", 128, 32), ("w_gl0", 65, 64),
    ("w_outed", 128, 32), ("w_hl0a", 64, 128), ("w_hl0b", 64, 128),
    ("w_nl1", 128, 64), ("nodesT", 64, 544), ("normn", 128, 256),
    ("t2mask", 64, 1024), ("s2mask", 32, 1024),
]:
    _WSEG[_name] = (_rows, _off, _cols)
    _off += _cols
WBLOB_COLS = _off
# scan tables in their own blob (big, consumed late)
_TSEG = {"scanmask": (128, 0, TG4), "nrm_gs": (128, TG4, TG4)}
TBLOB_COLS = 2 * TG4

_FSEG = {"ban1_t": (128, 0), "bae1_t": (128, 1), "ble1_t": (128, 2),
         "bln0_t": (128, 3), "bln1_t": (64, 4)}
FBLOB_COLS = 5


def _bf(x):
    return np.ascontiguousarray(np.asarray(x).astype(ml_dtypes.bfloat16))


def _f32(x):
    return np.ascontiguousarray(np.asarray(x).astype(np.float32))


# --------------------------------------------------------------------------
# static index tables (mirrors reference._build_indices structure)
# --------------------------------------------------------------------------
_t_i = np.minimum(np.arange(N), M)
_S = np.concatenate([[0], np.cumsum(_t_i)]).astype(np.int64)   # block starts
N_E = int(_S[N])                                               # 32240


def _core_tables(i0):
    li = np.arange(NL)
    gi = i0 + li
    tt = np.minimum(gi, 32)
    r = np.arange(32)
    real = r[None, :] >= (32 - tt[:, None])                    # [NL, 32]
    eidx = _S[gi][:, None] + (r[None, :] - (32 - tt[:, None]))  # [NL, 32]
    return real, eidx, tt


def _host_prep(b, h, inputs):
    """Build the per-core input map: xinit [97,T], wblob [128,WC], fblob."""
    i0 = 512 * h
    real, eidx, tt = _core_tables(i0)
    r = np.arange(32)
    li = np.arange(NL)

    nodes = _f32(inputs["input_nodes"][b])                     # [1024, 64]
    edges = _f32(inputs["input_edges"][b])                     # [N_E, 32]

    # xinit = [edgesT(0:32) | padmask(32) | node_j(33:97) | pad(97:128)]
    # 128 partitions so the HWDGE fans the transfer across all DMA engines
    xinit = np.zeros((128, T), np.float32)
    E_pad = np.zeros((NL, 32, 32), np.float32)
    E_pad[real] = edges[eidx[real]]
    xinit[0:32] = E_pad.reshape(T, 32).T
    xinit[32] = real.reshape(T)

    halo = np.zeros((544, 64), np.float32)
    jj = np.arange(i0 - 32, i0 + 512)
    halo[jj >= 0] = nodes[jj[jj >= 0]]
    nj = (li[:, None] + r[None, :]).reshape(T)
    xinit[33:97] = halo.T[:, nj]

    # weights
    Wan0, ban0 = inputs["Wan0"], inputs["ban0"]
    Wan1, ban1 = inputs["Wan1"], inputs["ban1"]
    Wln0, bln0 = inputs["Wln0"], inputs["bln0"]
    Wln1, bln1 = inputs["Wln1"], inputs["bln1"]
    Wae0, bae0 = inputs["Wae0"], inputs["bae0"]
    Wae1, bae1 = inputs["Wae1"], inputs["bae1"]
    Wle0, ble0 = inputs["Wle0"], inputs["ble0"]
    Wle1, ble1 = inputs["Wle1"], inputs["ble1"]

    tok = np.arange(T)
    m = (tok % 32 != 0).astype(np.float32)
    cnt = (r[None, :] - (32 - tt[:, None])).reshape(T).astype(np.float32)
    nrm = np.where(cnt > 0, 1.0 / np.maximum(cnt, 1), 1.0).astype(np.float32)
    norm_node = np.full(N, 1.0 / M, np.float32)
    norm_node[1:M] = 1.0 / np.arange(1, M)
    nv = norm_node[i0 + np.arange(NL)]

    seg = {
        "w_t1a": np.concatenate([np.asarray(Wan0)[64:96], np.asarray(ban0)[None, :],
                                 np.asarray(Wan0)[0:64]], 0),
        "w_t1b": np.asarray(Wan0)[96:160],
        "w_s1": np.concatenate([np.asarray(Wae0)[64:96], np.asarray(bae0)[None, :],
                                np.asarray(Wae0)[0:64]], 0),
        "w_t2": np.asarray(Wan1),
        "w_s2d": np.concatenate([np.asarray(Wae1), np.asarray(Wae1)], 0),
        "w_gl0": np.concatenate([np.asarray(Wle0)[32:64], np.asarray(ble0)[None, :],
                                 np.asarray(Wle0)[0:32]], 0),
        "w_outed": np.concatenate([np.asarray(Wle1), np.asarray(Wle1)], 0),
        "w_hl0a": np.asarray(Wln0)[0:64],
        "w_hl0b": np.asarray(Wln0)[64:128],
        "w_nl1": np.asarray(Wln1),
        "nodesT": halo.T,
        "normn": np.broadcast_to(nv.reshape(2, 256)[:, None, :],
                                 (2, 64, 256)).reshape(128, 256),
        "t2mask": real.reshape(T)[:1024][None, :].repeat(64, 0),
        "s2mask": real.reshape(T)[:1024][None, :].repeat(32, 0),
    }
    wblob = np.zeros((128, WBLOB_COLS), np.float32)
    for name, arr in seg.items():
        rows, off, cols = _WSEG[name]
        assert arr.shape == (rows, cols), (name, arr.shape)
        wblob[0:rows, off:off + cols] = arr

    tblob = np.zeros((128, TBLOB_COLS), np.float32)
    tblob[:, 0:TG4] = np.broadcast_to(m.reshape(4, TG4)[:, None, :],
                                      (4, 32, TG4)).reshape(128, TG4)
    tblob[:, TG4:] = np.broadcast_to(nrm.reshape(4, TG4)[:, None, :],
                                     (4, 32, TG4)).reshape(128, TG4)

    fblob = np.zeros((128, FBLOB_COLS), np.float32)
    fblob[0:128, 0] = np.concatenate([ban1, ban1])
    fblob[0:128, 1] = np.tile(np.asarray(bae1), 4)
    fblob[0:128, 2] = np.tile(np.asarray(ble1), 4)
    fblob[0:128, 3] = np.asarray(bln0)
    fblob[0:64, 4] = np.asarray(bln1)

    return {"xinit": _bf(xinit), "wblob": _bf(wblob), "tblob": _bf(tblob),
            "fblob": _f32(fblob)}


# --------------------------------------------------------------------------
# device kernel
# --------------------------------------------------------------------------
def build_kernel(debug=False):
    nc = bacc.Bacc("TRN2", target_bir_lowering=False, debug=debug)
    AF = mybir.ActivationFunctionType
    ALU = mybir.AluOpType

    xinit_d = nc.dram_tensor("xinit", [128, T], BF, kind="ExternalInput").ap()
    wblob_d = nc.dram_tensor("wblob", [128, WBLOB_COLS], BF, kind="ExternalInput").ap()
    tblob_d = nc.dram_tensor("tblob", [128, TBLOB_COLS], BF, kind="ExternalInput").ap()
    fblob_d = nc.dram_tensor("fblob", [128, FBLOB_COLS], F32, kind="ExternalInput").ap()
    out_n = nc.dram_tensor("out_n", [64, NL], F32, kind="ExternalOutput").ap()
    out_e = nc.dram_tensor("out_e", [128, TG4], BF, kind="ExternalOutput").ap()

    with tile.TileContext(nc) as tc:
        with (
            tc.tile_pool(name="big", bufs=1) as big,
            tc.tile_pool(name="pt1", bufs=2, space="PSUM") as pt1p,
            tc.tile_pool(name="ppk", bufs=4, space="PSUM") as ppk,
            tc.tile_pool(name="pnd", bufs=2, space="PSUM") as pnd,
        ):
            X = big.tile([128, T], BF, tag="X")
            t1 = big.tile([128, T], BF, tag="t1")
            S = big.tile([128, TG2], BF, tag="S")
            t2 = big.tile([128, TG2], BF, tag="t2")
            s2m = big.tile([128, TG4], BF, tag="s2m")
            incl = big.tile([128, TG4], BF, tag="incl")
            excl = big.tile([128, TG4], BF, tag="excl")
            Gg = big.tile([128, TG2], BF, tag="G")
            oute = big.tile([128, TG4], BF, tag="oute")
            W = big.tile([128, WBLOB_COLS], BF, tag="W")
            Tb = big.tile([128, TBLOB_COLS], BF, tag="Tb")
            Fb = big.tile([128, FBLOB_COLS], F32, tag="Fb")
            prevn_raw = big.tile([128, 256], F32, tag="prevn_raw")
            prevn = big.tile([128, 256], BF, tag="prevn")
            pn2 = big.tile([64, 512], BF, tag="pn2")
            hl0 = big.tile([128, 512], BF, tag="hl0")
            outn_sb = big.tile([64, 512], F32, tag="outn_sb")

            def w(name):
                rows, off, cols = _WSEG[name]
                return W[0:rows, off:off + cols]

            def tb(name):
                rows, off, cols = _TSEG[name]
                return Tb[0:rows, off:off + cols]

            def f(name):
                rows, col = _FSEG[name]
                return Fb[0:rows, col:col + 1]

            dma = nc.sync.dma_start
            dma(out=W[:, :], in_=wblob_d)
            dma(out=Fb[:, :], in_=fblob_d)
            for q in range(4):
                qs = slice(q * TG4, (q + 1) * TG4)
                dma(out=X[:, qs], in_=xinit_d[:, qs])
            dma(out=Tb[:, :], in_=tblob_d)

            def cw(c):
                return slice(c * NCH, (c + 1) * NCH)

            def node_i_bcast(c):
                # rhs [64, 16 blocks, 32 bcast] reading nodesT cols li+32
                base = w("nodesT")[:, 32 + 16 * c: 32 + 16 * c + 16]
                return bass.AP(tensor=base.tensor, offset=base.offset,
                               ap=[base.ap[0], base.ap[1], [0, 32]])

            # ---- t1 ------------------------------------------------------
            for c in range(NCHUNKS):
                pt = pt1p.tile([128, NCH], F32, tag="pt1")
                nc.tensor.matmul(pt[:, :], w("w_t1a"), X[0:97, cw(c)],
                                 start=True, stop=False)
                nc.tensor.matmul(pt[:, :], w("w_t1b"), node_i_bcast(c),
                                 start=False, stop=True)
                if c % 2 == 0:
                    nc.scalar.activation(t1[:, cw(c)], pt[:, :], AF.Relu)
                else:
                    nc.vector.tensor_scalar_max(t1[:, cw(c)], pt[:, :], 0.0)

            # ---- s1 (pack 2 chunks/psum tile) ---------------------------
            for cp in range(16):
                ps = ppk.tile([128, NCH], F32, tag="ppk")
                nc.tensor.matmul(ps[0:64, :], w("w_s1"), X[0:97, cw(cp)],
                                 start=True, stop=True)
                nc.tensor.matmul(ps[64:128, :], w("w_s1"), X[0:97, cw(cp + 16)],
                                 start=True, stop=True)
                if cp % 2 == 0:
                    nc.scalar.activation(S[:, cw(cp)], ps[:, :], AF.Relu)
                else:
                    nc.vector.tensor_scalar_max(S[:, cw(cp)], ps[:, :], 0.0)

            # ---- t2 (pack 2, ACT bias ban1) -----------------------------
            for cp in range(16):
                ps = ppk.tile([128, NCH], F32, tag="ppk")
                nc.tensor.matmul(ps[0:64, :], w("w_t2"), t1[:, cw(cp)],
                                 start=True, stop=True)
                nc.tensor.matmul(ps[64:128, :], w("w_t2"), t1[:, cw(cp + 16)],
                                 start=True, stop=True)
                nc.scalar.activation(t2[:, cw(cp)], ps[:, :], AF.Relu,
                                     bias=f("ban1_t"))

            # ---- node aggregation ---------------------------------------
            nc.vector.tensor_mul(t2[0:64, 0:1024], t2[0:64, 0:1024],
                                 w("t2mask"))
            t2v = t2[:, :].rearrange("p (b r) -> p b r", r=32)
            nc.vector.reduce_sum(out=prevn_raw[:, :], in_=t2v,
                                 axis=mybir.AxisListType.X)
            nc.vector.tensor_mul(prevn[:, :], prevn_raw[:, :], w("normn"))
            dma(out=pn2[0:64, 0:256], in_=prevn[0:64, :])
            dma(out=pn2[0:64, 256:512], in_=prevn[64:128, :])

            # ---- node MLP ------------------------------------------------
            ph = pnd.tile([128, 512], F32, tag="pnd")
            nc.tensor.matmul(ph[:, :], w("w_hl0a"), pn2[:, :],
                             start=True, stop=False)
            nc.tensor.matmul(ph[:, :], w("w_hl0b"), w("nodesT")[:, 32:544],
                             start=False, stop=True)
            nc.scalar.activation(hl0[:, :], ph[:, :], AF.Relu, bias=f("bln0_t"))
            po = pnd.tile([128, 512], F32, tag="pnd")
            nc.tensor.matmul(po[0:64, :], w("w_nl1"), hl0[:, :],
                             start=True, stop=True)
            nc.scalar.activation(outn_sb[:, :], po[0:64, :], AF.Relu,
                                 bias=f("bln1_t"))
            dma(out=out_n, in_=outn_sb[:, :])

            # ---- s2 (pack 4, ACT bias bae1) -----------------------------
            for cq in range(8):
                ps = ppk.tile([128, NCH], F32, tag="ppk")
                for k in range(4):
                    c = cq + 8 * k
                    g = c // 16
                    col = slice(c * NCH - g * TG2, (c + 1) * NCH - g * TG2)
                    nc.tensor.matmul(ps[32 * k:32 * k + 32, :],
                                     w("w_s2d")[64 * g:64 * g + 64, :],
                                     S[64 * g:64 * g + 64, col],
                                     start=True, stop=True,
                                     tile_position=(64 * g, 32 * k))
                nc.scalar.activation(s2m[:, cw(cq)], ps[:, :], AF.Relu,
                                     bias=f("bae1_t"))
            nc.vector.tensor_mul(s2m[0:32, 0:1024], s2m[0:32, 0:1024],
                                 w("s2mask"))

            # ---- edge prefix scan ---------------------------------------
            nc.vector.tensor_tensor_scan(incl[:, :], tb("scanmask"), s2m[:, :],
                                         0.0, ALU.mult, ALU.add)
            nc.vector.tensor_sub(excl[:, :], incl[:, :], s2m[:, :])
            nc.vector.tensor_mul(excl[:, :], excl[:, :], tb("nrm_gs"))
            for g in range(4):
                dma(out=X[33:65, g * TG4:(g + 1) * TG4],
                    in_=excl[32 * g:32 * g + 32, :])

            # ---- gl0 (pack 2) -------------------------------------------
            for cp in range(16):
                ps = ppk.tile([128, NCH], F32, tag="ppk")
                nc.tensor.matmul(ps[0:64, :], w("w_gl0"), X[0:65, cw(cp)],
                                 start=True, stop=True)
                nc.tensor.matmul(ps[64:128, :], w("w_gl0"), X[0:65, cw(cp + 16)],
                                 start=True, stop=True)
                if cp % 2 == 0:
                    nc.scalar.activation(Gg[:, cw(cp)], ps[:, :], AF.Relu)
                else:
                    nc.vector.tensor_scalar_max(Gg[:, cw(cp)], ps[:, :], 0.0)

            # ---- out_e (pack 4, ACT bias ble1) --------------------------
            for cq in range(8):
                ps = ppk.tile([128, NCH], F32, tag="ppk")
                for k in range(4):
                    c = cq + 8 * k
                    g = c // 16
                    col = slice(c * NCH - g * TG2, (c + 1) * NCH - g * TG2)
                    nc.tensor.matmul(ps[32 * k:32 * k + 32, :],
                                     w("w_outed")[64 * g:64 * g + 64, :],
                                     Gg[64 * g:64 * g + 64, col],
                                     start=True, stop=True,
                                     tile_position=(64 * g, 32 * k))
                nc.scalar.activation(oute[:, cw(cq)], ps[:, :], AF.Relu,
                                     bias=f("ble1_t"))
            dma(out=out_e, in_=oute[:, :])

    if not nc.is_finalized():
        nc.finalize()
    return nc


def _install_ntff_shim():
    """Provide antenv.axon_hooks (missing on this image) so trace=True can
    capture NTFF profiles via the axon .so C ABI. Only used when KERNEL_TRACE
    is set; the plain kernel() path never imports it."""
    import contextlib
    import ctypes
    import sys as _sys
    import types

    try:
        from antenv.axon_hooks import get_axon_ntff_profile_hook  # noqa: F401
        return
    except ImportError:
        pass

    so_path = "/opt/axon/libaxon_pjrt.so"
    hook = None
    try:
        lib = ctypes.CDLL(so_path)
        if hasattr(lib, "axon_start_nrt_profile"):
            lib.axon_start_nrt_profile.argtypes = [
                ctypes.POINTER(ctypes.c_int64), ctypes.c_size_t]
            lib.axon_start_nrt_profile.restype = ctypes.c_int64
            lib.axon_stop_nrt_profile.argtypes = [ctypes.c_char_p]
            lib.axon_stop_nrt_profile.restype = ctypes.c_int64

            @contextlib.contextmanager
            def _hook(output_dir, device_ids):
                import jax
                jax.devices()
                if device_ids:
                    ids = (ctypes.c_int64 * len(device_ids))(*device_ids)
                    rc = lib.axon_start_nrt_profile(ids, len(device_ids))
                else:
                    rc = lib.axon_start_nrt_profile(None, 0)
                if rc != 0:
                    raise RuntimeError(f"axon_start_nrt_profile rc={rc}")
                try:
                    yield
                finally:
                    n = lib.axon_stop_nrt_profile(str(output_dir).encode())
                    print(f"ntff profile: {n} file(s) -> {output_dir}")

            hook = _hook
    except OSError:
        pass

    mod = types.ModuleType("antenv.axon_hooks")
    mod._hook = hook
    mod.get_axon_ntff_profile_hook = lambda: mod._hook
    mod.set_axon_ntff_profile_hook = lambda h: setattr(mod, "_hook", h)
    import antenv
    antenv.axon_hooks = mod
    _sys.modules["antenv.axon_hooks"] = mod


# --------------------------------------------------------------------------
# host entry point
# --------------------------------------------------------------------------
def kernel(**inputs):
    in_maps = []
    metas = []
    for core in range(8):
        b, h = core // 2, core % 2
        in_maps.append(_host_prep(b, h, inputs))
        metas.append((b, h))

    nc = build_kernel(debug=False)
    trace = bool(os.environ.get("KERNEL_TRACE"))
    if trace:
        _install_ntff_shim()
    res = run_bass_kernel_spmd(nc, in_maps, core_ids=list(range(8)), trace=trace)
    global LAST_EXEC_NS
    LAST_EXEC_NS = res.exec_time_ns
    results = res.results

    output_nodes = np.zeros((B, N, 64), np.float32)
    output_edges = np.zeros((B, N_E, 32), np.float32)
    for core, (b, h) in enumerate(metas):
        i0 = 512 * h
        real, eidx, _ = _core_tables(i0)
        on = np.asarray(results[core]["out_n"], np.float32)         # [64, 512]
        oe = np.asarray(results[core]["out_e"]).astype(np.float32)  # [128, TG4]
        output_nodes[b, i0:i0 + NL] = on.T
        # unstack: partition p = f + 32*k, col c -> slot = k*TG4 + c
        oe4 = oe.reshape(4, 32, TG4)
        slots = np.transpose(oe4, (0, 2, 1)).reshape(T, 32)
        blk = slots.reshape(NL, 32, 32)
        output_edges[b][eidx[real]] = blk[real]
    return output_nodes, output_edges


# revision 24
# speedup vs baseline: 2.2873x; 1.1391x over previous
"""Trainium2 Bass kernel for nn_AutoRegressiveGraphConvLayer.

Self-contained: host-side layout prep (padding, transposes, bf16 cast, weight
stacking, mask/norm tables) + an 8-core SPMD Bass kernel + output reassembly.

Sharding: 8 cores = 4 batch samples x 2 node-range halves (nodes 0..511 /
512..1023). Each core handles T = 512*32 = 16384 padded edge slots (slot =
32*li + r; pad-at-start so the block structure is uniform) and 512 nodes.

Device pipeline (feature-major, bf16 operands, fp32 PSUM):
  X = [E(0:32) | padmask(32) | node_j(33:97)]   (one DMA; node_j im2col'd on host)
  t1 = relu(W_t1a.T @ X[0:97] + W_t1b.T @ node_i_bcast)        [128, T]
  t2 = relu(W_t2.T @ t1 + ban1)                 group-stacked   [128, T/2]
  prev_n = windowed reduce_sum(t2*mask) * norm  -> node MLP -> out_n
  s1 = relu(W_s1.T @ X[0:97])                   group-stacked   [128, T/2]
  s2 = relu(W_s2.T @ s1 + bae1)                 group-stacked   [128, T/4]
  prev_e = (segmented_scan(s2) - s2) * nrm      -> X[33:65]
  gl0 = relu(W_gl0.T @ X[0:65])                 group-stacked   [128, T/2]
  out_e = relu(W_oute.T @ gl0 + ble1)           group-stacked   [128, T/4]
All inputs arrive as 3 DRAM tensors per core (xinit, wblob bf16, fblob f32)
to keep per-instruction semaphore fan-in low.
"""

import os

import numpy as np
import ml_dtypes

import concourse.bass as bass
import concourse.tile as tile
from concourse import bacc, mybir
from concourse.bass_utils import run_bass_kernel_spmd

BF = mybir.dt.bfloat16
F32 = mybir.dt.float32

N, M, B = 1024, 32, 4
NL = 512                 # local nodes per core
T = NL * 32              # padded edge slots per core
NCH = 512                # matmul moving chunk
NCHUNKS = T // NCH       # 32
TG2, TG4 = T // 2, T // 4

LAST_EXEC_NS = None

# weight-blob column layout: name -> (row_count, col_offset, col_count)
_WSEG = {}
_off = 0
for _name, _rows, _cols in [
    ("w_t1a", 97, 128), ("w_t1b", 64, 128), ("w_s1", 97, 64),
    ("w_t2", 128, 64), ("w_s2d", 128, 32), ("w_gl0", 65, 64),
    ("w_outed", 128, 32), ("w_hl0a", 64, 128), ("w_hl0b", 64, 128),
    ("w_nl1", 128, 64), ("nodesT", 64, 544), ("normn", 128, 256),
    ("t2mask", 64, 1024), ("s2mask", 32, 1024),
]:
    _WSEG[_name] = (_rows, _off, _cols)
    _off += _cols
WBLOB_COLS = _off
# scan tables in their own blob (big, consumed late)
_TSEG = {"scanmask": (128, 0, TG4), "nrm_gs": (128, TG4, TG4)}
TBLOB_COLS = 2 * TG4

_FSEG = {"ban1_t": (128, 0), "bae1_t": (128, 1), "ble1_t": (128, 2),
         "bln0_t": (128, 3), "bln1_t": (64, 4)}
FBLOB_COLS = 5


def _bf(x):
    return np.ascontiguousarray(np.asarray(x).astype(ml_dtypes.bfloat16))


def _f32(x):
    return np.ascontiguousarray(np.asarray(x).astype(np.float32))


# --------------------------------------------------------------------------
# static index tables (mirrors reference._build_indices structure)
# --------------------------------------------------------------------------
_t_i = np.minimum(np.arange(N), M)
_S = np.concatenate([[0], np.cumsum(_t_i)]).astype(np.int64)   # block starts
N_E = int(_S[N])                                               # 32240


def _core_tables(i0):
    li = np.arange(NL)
    gi = i0 + li
    tt = np.minimum(gi, 32)
    r = np.arange(32)
    real = r[None, :] >= (32 - tt[:, None])                    # [NL, 32]
    eidx = _S[gi][:, None] + (r[None, :] - (32 - tt[:, None]))  # [NL, 32]
    return real, eidx, tt


def _host_prep(b, h, inputs):
    """Build the per-core input map: xinit [97,T], wblob [128,WC], fblob."""
    i0 = 512 * h
    real, eidx, tt = _core_tables(i0)
    r = np.arange(32)
    li = np.arange(NL)

    nodes = _f32(inputs["input_nodes"][b])                     # [1024, 64]
    edges = _f32(inputs["input_edges"][b])                     # [N_E, 32]

    # xinit = [edgesT(0:32) | padmask(32) | node_j(33:97) | pad(97:128)]
    # 128 partitions so the HWDGE fans the transfer across all DMA engines
    xinit = np.zeros((128, T), np.float32)
    E_pad = np.zeros((NL, 32, 32), np.float32)
    E_pad[real] = edges[eidx[real]]
    xinit[0:32] = E_pad.reshape(T, 32).T
    xinit[32] = real.reshape(T)

    halo = np.zeros((544, 64), np.float32)
    jj = np.arange(i0 - 32, i0 + 512)
    halo[jj >= 0] = nodes[jj[jj >= 0]]
    nj = (li[:, None] + r[None, :]).reshape(T)
    xinit[33:97] = halo.T[:, nj]

    # weights
    Wan0, ban0 = inputs["Wan0"], inputs["ban0"]
    Wan1, ban1 = inputs["Wan1"], inputs["ban1"]
    Wln0, bln0 = inputs["Wln0"], inputs["bln0"]
    Wln1, bln1 = inputs["Wln1"], inputs["bln1"]
    Wae0, bae0 = inputs["Wae0"], inputs["bae0"]
    Wae1, bae1 = inputs["Wae1"], inputs["bae1"]
    Wle0, ble0 = inputs["Wle0"], inputs["ble0"]
    Wle1, ble1 = inputs["Wle1"], inputs["ble1"]

    tok = np.arange(T)
    m = (tok % 32 != 0).astype(np.float32)
    cnt = (r[None, :] - (32 - tt[:, None])).reshape(T).astype(np.float32)
    nrm = np.where(cnt > 0, 1.0 / np.maximum(cnt, 1), 1.0).astype(np.float32)
    norm_node = np.full(N, 1.0 / M, np.float32)
    norm_node[1:M] = 1.0 / np.arange(1, M)
    nv = norm_node[i0 + np.arange(NL)]

    seg = {
        "w_t1a": np.concatenate([np.asarray(Wan0)[64:96], np.asarray(ban0)[None, :],
                                 np.asarray(Wan0)[0:64]], 0),
        "w_t1b": np.asarray(Wan0)[96:160],
        "w_s1": np.concatenate([np.asarray(Wae0)[64:96], np.asarray(bae0)[None, :],
                                np.asarray(Wae0)[0:64]], 0),
        "w_t2": np.asarray(Wan1),
        "w_s2d": np.concatenate([np.asarray(Wae1), np.asarray(Wae1)], 0),
        "w_gl0": np.concatenate([np.asarray(Wle0)[32:64], np.asarray(ble0)[None, :],
                                 np.asarray(Wle0)[0:32]], 0),
        "w_outed": np.concatenate([np.asarray(Wle1), np.asarray(Wle1)], 0),
        "w_hl0a": np.asarray(Wln0)[0:64],
        "w_hl0b": np.asarray(Wln0)[64:128],
        "w_nl1": np.asarray(Wln1),
        "nodesT": halo.T,
        "normn": np.broadcast_to(nv.reshape(2, 256)[:, None, :],
                                 (2, 64, 256)).reshape(128, 256),
        "t2mask": real.reshape(T)[:1024][None, :].repeat(64, 0),
        "s2mask": real.reshape(T)[:1024][None, :].repeat(32, 0),
    }
    wblob = np.zeros((128, WBLOB_COLS), np.float32)
    for name, arr in seg.items():
        rows, off, cols = _WSEG[name]
        assert arr.shape == (rows, cols), (name, arr.shape)
        wblob[0:rows, off:off + cols] = arr

    tblob = np.zeros((128, TBLOB_COLS), np.float32)
    tblob[:, 0:TG4] = np.broadcast_to(m.reshape(4, TG4)[:, None, :],
                                      (4, 32, TG4)).reshape(128, TG4)
    tblob[:, TG4:] = np.broadcast_to(nrm.reshape(4, TG4)[:, None, :],
                                     (4, 32, TG4)).reshape(128, TG4)

    fblob = np.zeros((128, FBLOB_COLS), np.float32)
    fblob[0:128, 0] = np.concatenate([ban1, ban1])
    fblob[0:128, 1] = np.tile(np.asarray(bae1), 4)
    fblob[0:128, 2] = np.tile(np.asarray(ble1), 4)
    fblob[0:128, 3] = np.asarray(bln0)
    fblob[0:64, 4] = np.asarray(bln1)

    return {"xinit": _bf(xinit), "wblob": _bf(wblob), "tblob": _bf(tblob),
            "fblob": _f32(fblob)}


# --------------------------------------------------------------------------
# device kernel
# --------------------------------------------------------------------------
def build_kernel(debug=False):
    nc = bacc.Bacc("TRN2", target_bir_lowering=False, debug=debug)
    AF = mybir.ActivationFunctionType
    ALU = mybir.AluOpType

    xinit_d = nc.dram_tensor("xinit", [128, T], BF, kind="ExternalInput").ap()
    wblob_d = nc.dram_tensor("wblob", [128, WBLOB_COLS], BF, kind="ExternalInput").ap()
    tblob_d = nc.dram_tensor("tblob", [128, TBLOB_COLS], BF, kind="ExternalInput").ap()
    fblob_d = nc.dram_tensor("fblob", [128, FBLOB_COLS], F32, kind="ExternalInput").ap()
    out_n = nc.dram_tensor("out_n", [64, NL], F32, kind="ExternalOutput").ap()
    out_e = nc.dram_tensor("out_e", [128, TG4], BF, kind="ExternalOutput").ap()

    with tile.TileContext(nc) as tc:
        with (
            tc.tile_pool(name="big", bufs=1) as big,
            tc.tile_pool(name="pt1", bufs=2, space="PSUM") as pt1p,
            tc.tile_pool(name="ppk", bufs=4, space="PSUM") as ppk,
            tc.tile_pool(name="pnd", bufs=2, space="PSUM") as pnd,
        ):
            X = big.tile([128, T], BF, tag="X")
            t1 = big.tile([128, T], BF, tag="t1")
            S = big.tile([128, TG2], BF, tag="S")
            t2 = big.tile([128, TG2], BF, tag="t2")
            s2m = big.tile([128, TG4], BF, tag="s2m")
            incl = big.tile([128, TG4], BF, tag="incl")
            excl = big.tile([128, TG4], BF, tag="excl")
            Gg = big.tile([128, TG2], BF, tag="G")
            oute = big.tile([128, TG4], BF, tag="oute")
            W = big.tile([128, WBLOB_COLS], BF, tag="W")
            Tb = big.tile([128, TBLOB_COLS], BF, tag="Tb")
            Fb = big.tile([128, FBLOB_COLS], F32, tag="Fb")
            prevn_raw = big.tile([128, 256], F32, tag="prevn_raw")
            prevn = big.tile([128, 256], BF, tag="prevn")
            pn2 = big.tile([64, 512], BF, tag="pn2")
            hl0 = big.tile([128, 512], BF, tag="hl0")
            outn_sb = big.tile([64, 512], F32, tag="outn_sb")

            def w(name):
                rows, off, cols = _WSEG[name]
                return W[0:rows, off:off + cols]

            def tb(name):
                rows, off, cols = _TSEG[name]
                return Tb[0:rows, off:off + cols]

            def f(name):
                rows, col = _FSEG[name]
                return Fb[0:rows, col:col + 1]

            dma = nc.sync.dma_start
            dma(out=W[:, :], in_=wblob_d)
            dma(out=Fb[:, :], in_=fblob_d)
            for q in range(4):
                qs = slice(q * TG4, (q + 1) * TG4)
                dma(out=X[:, qs], in_=xinit_d[:, qs])
            dma(out=Tb[:, :], in_=tblob_d)

            def cw(c):
                return slice(c * NCH, (c + 1) * NCH)

            def node_i_bcast(c):
                # rhs [64, 16 blocks, 32 bcast] reading nodesT cols li+32
                base = w("nodesT")[:, 32 + 16 * c: 32 + 16 * c + 16]
                return bass.AP(tensor=base.tensor, offset=base.offset,
                               ap=[base.ap[0], base.ap[1], [0, 32]])

            # ---- PE warmup during input load (HAM un-throttle) ----------
            for _ in range(12):
                pw = pnd.tile([128, 512], F32, tag="pnd")
                nc.tensor.matmul(pw[:, :], w("w_t1a")[:, 0:128],
                                 W[0:97, 0:512], start=True, stop=True)

            # ---- t1 ------------------------------------------------------
            for c in range(NCHUNKS):
                pt = pt1p.tile([128, NCH], F32, tag="pt1")
                nc.tensor.matmul(pt[:, :], w("w_t1a"), X[0:97, cw(c)],
                                 start=True, stop=False)
                nc.tensor.matmul(pt[:, :], w("w_t1b"), node_i_bcast(c),
                                 start=False, stop=True)
                if c % 2 == 0:
                    nc.scalar.activation(t1[:, cw(c)], pt[:, :], AF.Relu)
                else:
                    nc.vector.tensor_scalar_max(t1[:, cw(c)], pt[:, :], 0.0)

            # ---- s1 (pack 2 chunks/psum tile) ---------------------------
            for cp in range(16):
                ps = ppk.tile([128, NCH], F32, tag="ppk")
                nc.tensor.matmul(ps[0:64, :], w("w_s1"), X[0:97, cw(cp)],
                                 start=True, stop=True)
                nc.tensor.matmul(ps[64:128, :], w("w_s1"), X[0:97, cw(cp + 16)],
                                 start=True, stop=True)
                if cp % 2 == 0:
                    nc.scalar.activation(S[:, cw(cp)], ps[:, :], AF.Relu)
                else:
                    nc.vector.tensor_scalar_max(S[:, cw(cp)], ps[:, :], 0.0)

            # ---- s2 (pack 4, ACT bias bae1) -----------------------------
            for cq in range(8):
                ps = ppk.tile([128, NCH], F32, tag="ppk")
                for k in range(4):
                    c = cq + 8 * k
                    g = c // 16
                    col = slice(c * NCH - g * TG2, (c + 1) * NCH - g * TG2)
                    nc.tensor.matmul(ps[32 * k:32 * k + 32, :],
                                     w("w_s2d")[64 * g:64 * g + 64, :],
                                     S[64 * g:64 * g + 64, col],
                                     start=True, stop=True,
                                     tile_position=(64 * g, 32 * k))
                nc.scalar.activation(s2m[:, cw(cq)], ps[:, :], AF.Relu,
                                     bias=f("bae1_t"))
            nc.vector.tensor_mul(s2m[0:32, 0:1024], s2m[0:32, 0:1024],
                                 w("s2mask"))

            # ---- edge prefix scan ---------------------------------------
            nc.vector.tensor_tensor_scan(incl[:, :], tb("scanmask"), s2m[:, :],
                                         0.0, ALU.mult, ALU.add)
            nc.vector.tensor_sub(excl[:, :], incl[:, :], s2m[:, :])
            nc.vector.tensor_mul(excl[:, :], excl[:, :], tb("nrm_gs"))
            for g in range(4):
                dma(out=X[33:65, g * TG4:(g + 1) * TG4],
                    in_=excl[32 * g:32 * g + 32, :])

            # ---- t2 (pack 2, ACT bias ban1) — overlaps the scan chain ---
            for cp in range(16):
                ps = ppk.tile([128, NCH], F32, tag="ppk")
                nc.tensor.matmul(ps[0:64, :], w("w_t2"), t1[:, cw(cp)],
                                 start=True, stop=True)
                nc.tensor.matmul(ps[64:128, :], w("w_t2"), t1[:, cw(cp + 16)],
                                 start=True, stop=True)
                nc.scalar.activation(t2[:, cw(cp)], ps[:, :], AF.Relu,
                                     bias=f("ban1_t"))

            # ---- node aggregation (overlaps gl0 on PE) ------------------
            nc.vector.tensor_mul(t2[0:64, 0:1024], t2[0:64, 0:1024],
                                 w("t2mask"))
            t2v = t2[:, :].rearrange("p (b r) -> p b r", r=32)
            nc.vector.reduce_sum(out=prevn_raw[:, :], in_=t2v,
                                 axis=mybir.AxisListType.X)
            nc.vector.tensor_mul(prevn[:, :], prevn_raw[:, :], w("normn"))
            dma(out=pn2[0:64, 0:256], in_=prevn[0:64, :])
            dma(out=pn2[0:64, 256:512], in_=prevn[64:128, :])

            # ---- gl0 (pack 2) -------------------------------------------
            for cp in range(16):
                ps = ppk.tile([128, NCH], F32, tag="ppk")
                nc.tensor.matmul(ps[0:64, :], w("w_gl0"), X[0:65, cw(cp)],
                                 start=True, stop=True)
                nc.tensor.matmul(ps[64:128, :], w("w_gl0"), X[0:65, cw(cp + 16)],
                                 start=True, stop=True)
                if cp % 2 == 0:
                    nc.scalar.activation(Gg[:, cw(cp)], ps[:, :], AF.Relu)
                else:
                    nc.vector.tensor_scalar_max(Gg[:, cw(cp)], ps[:, :], 0.0)

            # ---- node MLP ------------------------------------------------
            ph = pnd.tile([128, 512], F32, tag="pnd")
            nc.tensor.matmul(ph[:, :], w("w_hl0a"), pn2[:, :],
                             start=True, stop=False)
            nc.tensor.matmul(ph[:, :], w("w_hl0b"), w("nodesT")[:, 32:544],
                             start=False, stop=True)
            nc.scalar.activation(hl0[:, :], ph[:, :], AF.Relu, bias=f("bln0_t"))
            po = pnd.tile([128, 512], F32, tag="pnd")
            nc.tensor.matmul(po[0:64, :], w("w_nl1"), hl0[:, :],
                             start=True, stop=True)
            nc.scalar.activation(outn_sb[:, :], po[0:64, :], AF.Relu,
                                 bias=f("bln1_t"))
            dma(out=out_n, in_=outn_sb[:, :])

            # ---- out_e (pack 4, ACT bias ble1) --------------------------
            for cq in range(8):
                ps = ppk.tile([128, NCH], F32, tag="ppk")
                for k in range(4):
                    c = cq + 8 * k
                    g = c // 16
                    col = slice(c * NCH - g * TG2, (c + 1) * NCH - g * TG2)
                    nc.tensor.matmul(ps[32 * k:32 * k + 32, :],
                                     w("w_outed")[64 * g:64 * g + 64, :],
                                     Gg[64 * g:64 * g + 64, col],
                                     start=True, stop=True,
                                     tile_position=(64 * g, 32 * k))
                nc.scalar.activation(oute[:, cw(cq)], ps[:, :], AF.Relu,
                                     bias=f("ble1_t"))
                if cq == 3:
                    dma(out=out_e[:, 0:4 * NCH], in_=oute[:, 0:4 * NCH])
            dma(out=out_e[:, 4 * NCH:], in_=oute[:, 4 * NCH:])

    if not nc.is_finalized():
        nc.finalize()
    return nc


def _install_ntff_shim():
    """Provide antenv.axon_hooks (missing on this image) so trace=True can
    capture NTFF profiles via the axon .so C ABI. Only used when KERNEL_TRACE
    is set; the plain kernel() path never imports it."""
    import contextlib
    import ctypes
    import sys as _sys
    import types

    try:
        from antenv.axon_hooks import get_axon_ntff_profile_hook  # noqa: F401
        return
    except ImportError:
        pass

    so_path = "/opt/axon/libaxon_pjrt.so"
    hook = None
    try:
        lib = ctypes.CDLL(so_path)
        if hasattr(lib, "axon_start_nrt_profile"):
            lib.axon_start_nrt_profile.argtypes = [
                ctypes.POINTER(ctypes.c_int64), ctypes.c_size_t]
            lib.axon_start_nrt_profile.restype = ctypes.c_int64
            lib.axon_stop_nrt_profile.argtypes = [ctypes.c_char_p]
            lib.axon_stop_nrt_profile.restype = ctypes.c_int64

            @contextlib.contextmanager
            def _hook(output_dir, device_ids):
                import jax
                jax.devices()
                if device_ids:
                    ids = (ctypes.c_int64 * len(device_ids))(*device_ids)
                    rc = lib.axon_start_nrt_profile(ids, len(device_ids))
                else:
                    rc = lib.axon_start_nrt_profile(None, 0)
                if rc != 0:
                    raise RuntimeError(f"axon_start_nrt_profile rc={rc}")
                try:
                    yield
                finally:
                    n = lib.axon_stop_nrt_profile(str(output_dir).encode())
                    print(f"ntff profile: {n} file(s) -> {output_dir}")

            hook = _hook
    except OSError:
        pass

    mod = types.ModuleType("antenv.axon_hooks")
    mod._hook = hook
    mod.get_axon_ntff_profile_hook = lambda: mod._hook
    mod.set_axon_ntff_profile_hook = lambda h: setattr(mod, "_hook", h)
    import antenv
    antenv.axon_hooks = mod
    _sys.modules["antenv.axon_hooks"] = mod


# --------------------------------------------------------------------------
# host entry point
# --------------------------------------------------------------------------
def kernel(**inputs):
    in_maps = []
    metas = []
    for core in range(8):
        b, h = core // 2, core % 2
        in_maps.append(_host_prep(b, h, inputs))
        metas.append((b, h))

    nc = build_kernel(debug=False)
    trace = bool(os.environ.get("KERNEL_TRACE"))
    if trace:
        _install_ntff_shim()
    res = run_bass_kernel_spmd(nc, in_maps, core_ids=list(range(8)), trace=trace)
    global LAST_EXEC_NS
    LAST_EXEC_NS = res.exec_time_ns
    results = res.results

    output_nodes = np.zeros((B, N, 64), np.float32)
    output_edges = np.zeros((B, N_E, 32), np.float32)
    for core, (b, h) in enumerate(metas):
        i0 = 512 * h
        real, eidx, _ = _core_tables(i0)
        on = np.asarray(results[core]["out_n"], np.float32)         # [64, 512]
        oe = np.asarray(results[core]["out_e"]).astype(np.float32)  # [128, TG4]
        output_nodes[b, i0:i0 + NL] = on.T
        # unstack: partition p = f + 32*k, col c -> slot = k*TG4 + c
        oe4 = oe.reshape(4, 32, TG4)
        slots = np.transpose(oe4, (0, 2, 1)).reshape(T, 32)
        blk = slots.reshape(NL, 32, 32)
        output_edges[b][eidx[real]] = blk[real]
    return output_nodes, output_edges


# revision 27
# speedup vs baseline: 2.7220x; 1.1900x over previous
"""Trainium2 Bass kernel for nn_AutoRegressiveGraphConvLayer.

Self-contained: host-side layout prep (padding, transposes, bf16 cast, weight
stacking, mask/norm tables) + an 8-core SPMD Bass kernel + output reassembly.

Sharding: 8 cores = 4 batch samples x 2 node-range halves (nodes 0..511 /
512..1023). Each core handles T = 512*32 = 16384 padded edge slots (slot =
32*li + r; pad-at-start so the block structure is uniform) and 512 nodes.

Device pipeline (feature-major, bf16 operands, fp32 PSUM):
  X = [E(0:32) | padmask(32) | node_j(33:97)]   (one DMA; node_j im2col'd on host)
  t1 = relu(W_t1a.T @ X[0:97] + W_t1b.T @ node_i_bcast)        [128, T]
  t2 = relu(W_t2.T @ t1 + ban1)                 group-stacked   [128, T/2]
  prev_n = windowed reduce_sum(t2*mask) * norm  -> node MLP -> out_n
  s1 = relu(W_s1.T @ X[0:97])                   group-stacked   [128, T/2]
  s2 = relu(W_s2.T @ s1 + bae1)                 group-stacked   [128, T/4]
  prev_e = (segmented_scan(s2) - s2) * nrm      -> X[33:65]
  gl0 = relu(W_gl0.T @ X[0:65])                 group-stacked   [128, T/2]
  out_e = relu(W_oute.T @ gl0 + ble1)           group-stacked   [128, T/4]
All inputs arrive as 3 DRAM tensors per core (xinit, wblob bf16, fblob f32)
to keep per-instruction semaphore fan-in low.
"""

import os

import numpy as np
import ml_dtypes

import concourse.bass as bass
import concourse.tile as tile
from concourse import bacc, mybir
from concourse.bass_utils import run_bass_kernel_spmd

BF = mybir.dt.bfloat16
F32 = mybir.dt.float32

N, M, B = 1024, 32, 4
NL = 512                 # local nodes per core
T = NL * 32              # padded edge slots per core
NCH = 512                # matmul moving chunk
NCHUNKS = T // NCH       # 32
TG2, TG4 = T // 2, T // 4

LAST_EXEC_NS = None

# weight-blob column layout: name -> (row_count, col_offset, col_count)
_WSEG = {}
_off = 0
for _name, _rows, _cols in [
    ("w_t1a", 128, 128), ("w_t1b", 64, 128), ("w_s1", 128, 64),
    ("w_t2", 128, 64), ("w_s2d", 128, 32), ("w_gl0", 128, 64),
    ("w_outed", 128, 32), ("w_hl0a", 64, 128), ("w_hl0b", 64, 128),
    ("w_nl1", 128, 64), ("nodesT", 64, 544), ("normn", 128, 256),
    ("t2mask", 64, 1024), ("s2mask", 32, 1024),
]:
    _WSEG[_name] = (_rows, _off, _cols)
    _off += _cols
WBLOB_COLS = _off
# scan tables in their own blob (big, consumed late)
_TSEG = {"scanmask": (128, 0, TG4), "nrm_gs": (128, TG4, TG4)}
TBLOB_COLS = 2 * TG4

_FSEG = {"ban1_t": (128, 0), "bae1_t": (128, 1), "ble1_t": (128, 2),
         "bln0_t": (128, 3), "bln1_t": (64, 4)}
FBLOB_COLS = 5


def _bf(x):
    return np.ascontiguousarray(np.asarray(x).astype(ml_dtypes.bfloat16))


def _f32(x):
    return np.ascontiguousarray(np.asarray(x).astype(np.float32))


# --------------------------------------------------------------------------
# static index tables (mirrors reference._build_indices structure)
# --------------------------------------------------------------------------
_t_i = np.minimum(np.arange(N), M)
_S = np.concatenate([[0], np.cumsum(_t_i)]).astype(np.int64)   # block starts
N_E = int(_S[N])                                               # 32240


def _core_tables(i0):
    li = np.arange(NL)
    gi = i0 + li
    tt = np.minimum(gi, 32)
    r = np.arange(32)
    real = r[None, :] >= (32 - tt[:, None])                    # [NL, 32]
    eidx = _S[gi][:, None] + (r[None, :] - (32 - tt[:, None]))  # [NL, 32]
    return real, eidx, tt


def _host_prep(b, h, inputs):
    """Build the per-core input map: xinit [97,T], wblob [128,WC], fblob."""
    i0 = 512 * h
    real, eidx, tt = _core_tables(i0)
    r = np.arange(32)
    li = np.arange(NL)

    nodes = _f32(inputs["input_nodes"][b])                     # [1024, 64]
    edges = _f32(inputs["input_edges"][b])                     # [N_E, 32]

    # xinit = [edgesT(0:32) | padmask(32) | node_j(33:97) | pad(97:128)]
    # 128 partitions so the HWDGE fans the transfer across all DMA engines
    xinit = np.zeros((128, T), np.float32)
    E_pad = np.zeros((NL, 32, 32), np.float32)
    E_pad[real] = edges[eidx[real]]
    xinit[0:32] = E_pad.reshape(T, 32).T
    xinit[32] = real.reshape(T)

    halo = np.zeros((544, 64), np.float32)
    jj = np.arange(i0 - 32, i0 + 512)
    halo[jj >= 0] = nodes[jj[jj >= 0]]
    nj = (li[:, None] + r[None, :]).reshape(T)
    xinit[33:97] = halo.T[:, nj]

    # weights
    Wan0, ban0 = inputs["Wan0"], inputs["ban0"]
    Wan1, ban1 = inputs["Wan1"], inputs["ban1"]
    Wln0, bln0 = inputs["Wln0"], inputs["bln0"]
    Wln1, bln1 = inputs["Wln1"], inputs["bln1"]
    Wae0, bae0 = inputs["Wae0"], inputs["bae0"]
    Wae1, bae1 = inputs["Wae1"], inputs["bae1"]
    Wle0, ble0 = inputs["Wle0"], inputs["ble0"]
    Wle1, ble1 = inputs["Wle1"], inputs["ble1"]

    tok = np.arange(T)
    m = (tok % 32 != 0).astype(np.float32)
    cnt = (r[None, :] - (32 - tt[:, None])).reshape(T).astype(np.float32)
    nrm = np.where(cnt > 0, 1.0 / np.maximum(cnt, 1), 1.0).astype(np.float32)
    norm_node = np.full(N, 1.0 / M, np.float32)
    norm_node[1:M] = 1.0 / np.arange(1, M)
    nv = norm_node[i0 + np.arange(NL)]

    z31x128 = np.zeros((31, 128), np.float32)
    z31x64 = np.zeros((31, 64), np.float32)
    seg = {
        # K padded to 128 (zero rows against zeroed/ignored X rows) for FWL
        "w_t1a": np.concatenate([np.asarray(Wan0)[64:96], np.asarray(ban0)[None, :],
                                 np.asarray(Wan0)[0:64], z31x128], 0),
        "w_t1b": np.asarray(Wan0)[96:160],
        "w_s1": np.concatenate([np.asarray(Wae0)[64:96], np.asarray(bae0)[None, :],
                                np.asarray(Wae0)[0:64], z31x64], 0),
        "w_t2": np.asarray(Wan1),
        "w_s2d": np.concatenate([np.asarray(Wae1), np.asarray(Wae1)], 0),
        "w_gl0": np.concatenate([np.asarray(Wle0)[32:64], np.asarray(ble0)[None, :],
                                 np.asarray(Wle0)[0:32],
                                 np.zeros((63, 64), np.float32)], 0),
        "w_outed": np.concatenate([np.asarray(Wle1), np.asarray(Wle1)], 0),
        "w_hl0a": np.asarray(Wln0)[0:64],
        "w_hl0b": np.asarray(Wln0)[64:128],
        "w_nl1": np.asarray(Wln1),
        "nodesT": halo.T,
        "normn": np.broadcast_to(nv.reshape(2, 256)[:, None, :],
                                 (2, 64, 256)).reshape(128, 256),
        "t2mask": real.reshape(T)[:1024][None, :].repeat(64, 0),
        "s2mask": real.reshape(T)[:1024][None, :].repeat(32, 0),
    }
    wblob = np.zeros((128, WBLOB_COLS), np.float32)
    for name, arr in seg.items():
        rows, off, cols = _WSEG[name]
        assert arr.shape == (rows, cols), (name, arr.shape)
        wblob[0:rows, off:off + cols] = arr

    tblob = np.zeros((128, TBLOB_COLS), np.float32)
    tblob[:, 0:TG4] = np.broadcast_to(m.reshape(4, TG4)[:, None, :],
                                      (4, 32, TG4)).reshape(128, TG4)
    tblob[:, TG4:] = np.broadcast_to(nrm.reshape(4, TG4)[:, None, :],
                                     (4, 32, TG4)).reshape(128, TG4)

    fblob = np.zeros((128, FBLOB_COLS), np.float32)
    fblob[0:128, 0] = np.concatenate([ban1, ban1])
    fblob[0:128, 1] = np.tile(np.asarray(bae1), 4)
    fblob[0:128, 2] = np.tile(np.asarray(ble1), 4)
    fblob[0:128, 3] = np.asarray(bln0)
    fblob[0:64, 4] = np.asarray(bln1)

    return {"xinit": _bf(xinit), "wblob": _bf(wblob), "tblob": _bf(tblob),
            "fblob": _f32(fblob)}


# --------------------------------------------------------------------------
# device kernel
# --------------------------------------------------------------------------
def build_kernel(debug=False):
    nc = bacc.Bacc("TRN2", target_bir_lowering=False, debug=debug)
    AF = mybir.ActivationFunctionType
    ALU = mybir.AluOpType

    xinit_d = nc.dram_tensor("xinit", [128, T], BF, kind="ExternalInput").ap()
    wblob_d = nc.dram_tensor("wblob", [128, WBLOB_COLS], BF, kind="ExternalInput").ap()
    tblob_d = nc.dram_tensor("tblob", [128, TBLOB_COLS], BF, kind="ExternalInput").ap()
    fblob_d = nc.dram_tensor("fblob", [128, FBLOB_COLS], F32, kind="ExternalInput").ap()
    out_n = nc.dram_tensor("out_n", [64, NL], F32, kind="ExternalOutput").ap()
    out_e = nc.dram_tensor("out_e", [128, TG4], BF, kind="ExternalOutput").ap()

    with tile.TileContext(nc) as tc:
        with (
            tc.tile_pool(name="big", bufs=1) as big,
            tc.tile_pool(name="pt1", bufs=2, space="PSUM") as pt1p,
            tc.tile_pool(name="ppk", bufs=4, space="PSUM") as ppk,
            tc.tile_pool(name="pnd", bufs=2, space="PSUM") as pnd,
        ):
            X = big.tile([128, T], BF, tag="X")
            t1 = big.tile([128, T], BF, tag="t1")
            S = big.tile([128, TG2], BF, tag="S")
            t2 = big.tile([128, TG2], BF, tag="t2")
            s2m = big.tile([128, TG4], BF, tag="s2m")
            incl = big.tile([128, TG4], BF, tag="incl")
            excl = big.tile([128, TG4], BF, tag="excl")
            Gg = big.tile([128, TG2], BF, tag="G")
            oute = big.tile([128, TG4], BF, tag="oute")
            W = big.tile([128, WBLOB_COLS], BF, tag="W")
            Tb = big.tile([128, TBLOB_COLS], BF, tag="Tb")
            Fb = big.tile([128, FBLOB_COLS], F32, tag="Fb")
            prevn_raw = big.tile([128, 256], F32, tag="prevn_raw")
            prevn = big.tile([128, 256], BF, tag="prevn")
            pn2 = big.tile([64, 512], BF, tag="pn2")
            hl0 = big.tile([128, 512], BF, tag="hl0")
            outn_sb = big.tile([64, 512], F32, tag="outn_sb")

            def w(name):
                rows, off, cols = _WSEG[name]
                return W[0:rows, off:off + cols]

            def tb(name):
                rows, off, cols = _TSEG[name]
                return Tb[0:rows, off:off + cols]

            def f(name):
                rows, col = _FSEG[name]
                return Fb[0:rows, col:col + 1]

            dma = nc.sync.dma_start
            dma(out=W[:, :], in_=wblob_d)
            dma(out=Fb[:, :], in_=fblob_d)
            for q in range(4):
                qs = slice(q * TG4, (q + 1) * TG4)
                dma(out=X[:, qs], in_=xinit_d[:, qs])
            dma(out=Tb[:, :], in_=tblob_d)

            def cw(c):
                return slice(c * NCH, (c + 1) * NCH)

            def node_i_bcast(c):
                # rhs [64, 16 blocks, 32 bcast] reading nodesT cols li+32
                base = w("nodesT")[:, 32 + 16 * c: 32 + 16 * c + 16]
                return bass.AP(tensor=base.tensor, offset=base.offset,
                               ap=[base.ap[0], base.ap[1], [0, 32]])

            # ---- PE warmup during input load (HAM un-throttle) ----------
            for _ in range(12):
                pw = pnd.tile([128, 512], F32, tag="pnd")
                nc.tensor.matmul(pw[:, :], w("w_t1a")[:, 0:128],
                                 W[0:128, 0:512], start=True, stop=True)

            # ---- t1 ------------------------------------------------------
            for c in range(NCHUNKS):
                pt = pt1p.tile([128, NCH], F32, tag="pt1")
                nc.tensor.matmul(pt[:, :], w("w_t1a"), X[0:128, cw(c)],
                                 start=True, stop=False)
                nc.tensor.matmul(pt[:, :], w("w_t1b"), node_i_bcast(c),
                                 start=False, stop=True)
                if c % 2 == 0:
                    nc.scalar.activation(t1[:, cw(c)], pt[:, :], AF.Relu)
                else:
                    nc.vector.tensor_scalar_max(t1[:, cw(c)], pt[:, :], 0.0)

            # ---- s1 (pack 2 chunks/psum tile) ---------------------------
            for cp in range(16):
                ps = ppk.tile([128, NCH], F32, tag="ppk")
                nc.tensor.matmul(ps[0:64, :], w("w_s1"), X[0:128, cw(cp)],
                                 start=True, stop=True)
                nc.tensor.matmul(ps[64:128, :], w("w_s1"), X[0:128, cw(cp + 16)],
                                 start=True, stop=True)
                if cp % 2 == 0:
                    nc.scalar.activation(S[:, cw(cp)], ps[:, :], AF.Relu)
                else:
                    nc.vector.tensor_scalar_max(S[:, cw(cp)], ps[:, :], 0.0)

            # ---- s2 (pack 4, ACT bias bae1) -----------------------------
            for cq in range(8):
                ps = ppk.tile([128, NCH], F32, tag="ppk")
                for k in range(4):
                    c = cq + 8 * k
                    g = c // 16
                    col = slice(c * NCH - g * TG2, (c + 1) * NCH - g * TG2)
                    nc.tensor.matmul(ps[32 * k:32 * k + 32, :],
                                     w("w_s2d")[64 * g:64 * g + 64, :],
                                     S[64 * g:64 * g + 64, col],
                                     start=True, stop=True,
                                     tile_position=(64 * g, 32 * k))
                nc.scalar.activation(s2m[:, cw(cq)], ps[:, :], AF.Relu,
                                     bias=f("bae1_t"))
            nc.vector.tensor_mul(s2m[0:32, 0:1024], s2m[0:32, 0:1024],
                                 w("s2mask"))

            # ---- edge prefix scan ---------------------------------------
            nc.vector.tensor_tensor_scan(incl[:, :], tb("scanmask"), s2m[:, :],
                                         0.0, ALU.mult, ALU.add)
            nc.vector.tensor_sub(excl[:, :], incl[:, :], s2m[:, :])
            nc.vector.tensor_mul(excl[:, :], excl[:, :], tb("nrm_gs"))
            for g in range(4):
                dma(out=X[33:65, g * TG4:(g + 1) * TG4],
                    in_=excl[32 * g:32 * g + 32, :])

            # ---- t2 (pack 2, ACT bias ban1) — overlaps the scan chain ---
            for cp in range(16):
                ps = ppk.tile([128, NCH], F32, tag="ppk")
                nc.tensor.matmul(ps[0:64, :], w("w_t2"), t1[:, cw(cp)],
                                 start=True, stop=True)
                nc.tensor.matmul(ps[64:128, :], w("w_t2"), t1[:, cw(cp + 16)],
                                 start=True, stop=True)
                nc.scalar.activation(t2[:, cw(cp)], ps[:, :], AF.Relu,
                                     bias=f("ban1_t"))

            # ---- node aggregation (overlaps gl0 on PE) ------------------
            nc.vector.tensor_mul(t2[0:64, 0:1024], t2[0:64, 0:1024],
                                 w("t2mask"))
            t2v = t2[:, :].rearrange("p (b r) -> p b r", r=32)
            nc.vector.reduce_sum(out=prevn_raw[:, :], in_=t2v,
                                 axis=mybir.AxisListType.X)
            nc.vector.tensor_mul(prevn[:, :], prevn_raw[:, :], w("normn"))
            dma(out=pn2[0:64, 0:256], in_=prevn[0:64, :])
            dma(out=pn2[0:64, 256:512], in_=prevn[64:128, :])

            # ---- gl0 (pack 2) -------------------------------------------
            for cp in range(16):
                ps = ppk.tile([128, NCH], F32, tag="ppk")
                nc.tensor.matmul(ps[0:64, :], w("w_gl0"), X[0:128, cw(cp)],
                                 start=True, stop=True)
                nc.tensor.matmul(ps[64:128, :], w("w_gl0"), X[0:128, cw(cp + 16)],
                                 start=True, stop=True)
                if cp % 2 == 0:
                    nc.scalar.activation(Gg[:, cw(cp)], ps[:, :], AF.Relu)
                else:
                    nc.vector.tensor_scalar_max(Gg[:, cw(cp)], ps[:, :], 0.0)

            # ---- node MLP ------------------------------------------------
            ph = pnd.tile([128, 512], F32, tag="pnd")
            nc.tensor.matmul(ph[:, :], w("w_hl0a"), pn2[:, :],
                             start=True, stop=False)
            nc.tensor.matmul(ph[:, :], w("w_hl0b"), w("nodesT")[:, 32:544],
                             start=False, stop=True)
            nc.scalar.activation(hl0[:, :], ph[:, :], AF.Relu, bias=f("bln0_t"))
            po = pnd.tile([128, 512], F32, tag="pnd")
            nc.tensor.matmul(po[0:64, :], w("w_nl1"), hl0[:, :],
                             start=True, stop=True)
            nc.scalar.activation(outn_sb[:, :], po[0:64, :], AF.Relu,
                                 bias=f("bln1_t"))
            dma(out=out_n, in_=outn_sb[:, :])

            # ---- out_e (pack 4, ACT bias ble1) --------------------------
            for cq in range(8):
                ps = ppk.tile([128, NCH], F32, tag="ppk")
                for k in range(4):
                    c = cq + 8 * k
                    g = c // 16
                    col = slice(c * NCH - g * TG2, (c + 1) * NCH - g * TG2)
                    nc.tensor.matmul(ps[32 * k:32 * k + 32, :],
                                     w("w_outed")[64 * g:64 * g + 64, :],
                                     Gg[64 * g:64 * g + 64, col],
                                     start=True, stop=True,
                                     tile_position=(64 * g, 32 * k))
                nc.scalar.activation(oute[:, cw(cq)], ps[:, :], AF.Relu,
                                     bias=f("ble1_t"))
                if cq == 3:
                    dma(out=out_e[:, 0:4 * NCH], in_=oute[:, 0:4 * NCH])
            dma(out=out_e[:, 4 * NCH:], in_=oute[:, 4 * NCH:])

    if not nc.is_finalized():
        nc.finalize()
    return nc


def _install_ntff_shim():
    """Provide antenv.axon_hooks (missing on this image) so trace=True can
    capture NTFF profiles via the axon .so C ABI. Only used when KERNEL_TRACE
    is set; the plain kernel() path never imports it."""
    import contextlib
    import ctypes
    import sys as _sys
    import types

    try:
        from antenv.axon_hooks import get_axon_ntff_profile_hook  # noqa: F401
        return
    except ImportError:
        pass

    so_path = "/opt/axon/libaxon_pjrt.so"
    hook = None
    try:
        lib = ctypes.CDLL(so_path)
        if hasattr(lib, "axon_start_nrt_profile"):
            lib.axon_start_nrt_profile.argtypes = [
                ctypes.POINTER(ctypes.c_int64), ctypes.c_size_t]
            lib.axon_start_nrt_profile.restype = ctypes.c_int64
            lib.axon_stop_nrt_profile.argtypes = [ctypes.c_char_p]
            lib.axon_stop_nrt_profile.restype = ctypes.c_int64

            @contextlib.contextmanager
            def _hook(output_dir, device_ids):
                import jax
                jax.devices()
                if device_ids:
                    ids = (ctypes.c_int64 * len(device_ids))(*device_ids)
                    rc = lib.axon_start_nrt_profile(ids, len(device_ids))
                else:
                    rc = lib.axon_start_nrt_profile(None, 0)
                if rc != 0:
                    raise RuntimeError(f"axon_start_nrt_profile rc={rc}")
                try:
                    yield
                finally:
                    n = lib.axon_stop_nrt_profile(str(output_dir).encode())
                    print(f"ntff profile: {n} file(s) -> {output_dir}")

            hook = _hook
    except OSError:
        pass

    mod = types.ModuleType("antenv.axon_hooks")
    mod._hook = hook
    mod.get_axon_ntff_profile_hook = lambda: mod._hook
    mod.set_axon_ntff_profile_hook = lambda h: setattr(mod, "_hook", h)
    import antenv
    antenv.axon_hooks = mod
    _sys.modules["antenv.axon_hooks"] = mod


# --------------------------------------------------------------------------
# host entry point
# --------------------------------------------------------------------------
def kernel(**inputs):
    in_maps = []
    metas = []
    for core in range(8):
        b, h = core // 2, core % 2
        in_maps.append(_host_prep(b, h, inputs))
        metas.append((b, h))

    nc = build_kernel(debug=False)
    trace = bool(os.environ.get("KERNEL_TRACE"))
    if trace:
        _install_ntff_shim()
    res = run_bass_kernel_spmd(nc, in_maps, core_ids=list(range(8)), trace=trace)
    global LAST_EXEC_NS
    LAST_EXEC_NS = res.exec_time_ns
    results = res.results

    output_nodes = np.zeros((B, N, 64), np.float32)
    output_edges = np.zeros((B, N_E, 32), np.float32)
    for core, (b, h) in enumerate(metas):
        i0 = 512 * h
        real, eidx, _ = _core_tables(i0)
        on = np.asarray(results[core]["out_n"], np.float32)         # [64, 512]
        oe = np.asarray(results[core]["out_e"]).astype(np.float32)  # [128, TG4]
        output_nodes[b, i0:i0 + NL] = on.T
        # unstack: partition p = f + 32*k, col c -> slot = k*TG4 + c
        oe4 = oe.reshape(4, 32, TG4)
        slots = np.transpose(oe4, (0, 2, 1)).reshape(T, 32)
        blk = slots.reshape(NL, 32, 32)
        output_edges[b][eidx[real]] = blk[real]
    return output_nodes, output_edges


# revision 29
# speedup vs baseline: 2.8139x; 1.0338x over previous
"""Trainium2 Bass kernel for nn_AutoRegressiveGraphConvLayer.

Self-contained: host-side layout prep (padding, transposes, bf16 cast, weight
stacking, mask/norm tables) + an 8-core SPMD Bass kernel + output reassembly.

Sharding: 8 cores = 4 batch samples x 2 node-range halves (nodes 0..511 /
512..1023). Each core handles T = 512*32 = 16384 padded edge slots (slot =
32*li + r; pad-at-start so the block structure is uniform) and 512 nodes.

Device pipeline (feature-major, bf16 operands, fp32 PSUM):
  X = [E(0:32) | padmask(32) | node_j(33:97)]   (one DMA; node_j im2col'd on host)
  t1 = relu(W_t1a.T @ X[0:97] + W_t1b.T @ node_i_bcast)        [128, T]
  t2 = relu(W_t2.T @ t1 + ban1)                 group-stacked   [128, T/2]
  prev_n = windowed reduce_sum(t2*mask) * norm  -> node MLP -> out_n
  s1 = relu(W_s1.T @ X[0:97])                   group-stacked   [128, T/2]
  s2 = relu(W_s2.T @ s1 + bae1)                 group-stacked   [128, T/4]
  prev_e = (segmented_scan(s2) - s2) * nrm      -> X[33:65]
  gl0 = relu(W_gl0.T @ X[0:65])                 group-stacked   [128, T/2]
  out_e = relu(W_oute.T @ gl0 + ble1)           group-stacked   [128, T/4]
All inputs arrive as 3 DRAM tensors per core (xinit, wblob bf16, fblob f32)
to keep per-instruction semaphore fan-in low.
"""

import os

import numpy as np
import ml_dtypes

import concourse.bass as bass
import concourse.tile as tile
from concourse import bacc, mybir
from concourse.bass_utils import run_bass_kernel_spmd

BF = mybir.dt.bfloat16
F32 = mybir.dt.float32

N, M, B = 1024, 32, 4
NL = 512                 # local nodes per core
T = NL * 32              # padded edge slots per core
NCH = 512                # matmul moving chunk
NCHUNKS = T // NCH       # 32
TG2, TG4 = T // 2, T // 4

LAST_EXEC_NS = None

# weight-blob column layout: name -> (row_count, col_offset, col_count)
_WSEG = {}
_off = 0
for _name, _rows, _cols in [
    ("w_t1a", 128, 128), ("w_t1b", 64, 128), ("w_s1", 128, 64),
    ("w_t2", 128, 64), ("w_s2d", 128, 32), ("w_gl0", 128, 64),
    ("w_outed", 128, 32), ("w_hl0a", 64, 128), ("w_hl0b", 64, 128),
    ("w_nl1", 128, 64), ("nodesT", 64, 544), ("normn", 128, 256),
    ("t2mask", 64, 1024), ("s2mask", 32, 1024),
]:
    _WSEG[_name] = (_rows, _off, _cols)
    _off += _cols
WBLOB_COLS = _off
# scan tables in their own blob (big, consumed late)
_TSEG = {"scanmask": (128, 0, TG4), "nrm_gs": (128, TG4, TG4)}
TBLOB_COLS = 2 * TG4

_FSEG = {"ban1_t": (128, 0), "bae1_t": (128, 1), "ble1_t": (128, 2),
         "bln0_t": (128, 3), "bln1_t": (64, 4)}
FBLOB_COLS = 5


def _bf(x):
    return np.ascontiguousarray(np.asarray(x).astype(ml_dtypes.bfloat16))


def _f32(x):
    return np.ascontiguousarray(np.asarray(x).astype(np.float32))


# --------------------------------------------------------------------------
# static index tables (mirrors reference._build_indices structure)
# --------------------------------------------------------------------------
_t_i = np.minimum(np.arange(N), M)
_S = np.concatenate([[0], np.cumsum(_t_i)]).astype(np.int64)   # block starts
N_E = int(_S[N])                                               # 32240


def _core_tables(i0):
    li = np.arange(NL)
    gi = i0 + li
    tt = np.minimum(gi, 32)
    r = np.arange(32)
    real = r[None, :] >= (32 - tt[:, None])                    # [NL, 32]
    eidx = _S[gi][:, None] + (r[None, :] - (32 - tt[:, None]))  # [NL, 32]
    return real, eidx, tt


def _host_prep(b, h, inputs):
    """Build the per-core input map: xinit [97,T], wblob [128,WC], fblob."""
    i0 = 512 * h
    real, eidx, tt = _core_tables(i0)
    r = np.arange(32)
    li = np.arange(NL)

    nodes = _f32(inputs["input_nodes"][b])                     # [1024, 64]
    edges = _f32(inputs["input_edges"][b])                     # [N_E, 32]

    # xinit = [edgesT(0:32) | padmask(32) | node_j(33:97) | pad(97:128)]
    # 128 partitions so the HWDGE fans the transfer across all DMA engines
    xinit = np.zeros((128, T), np.float32)
    E_pad = np.zeros((NL, 32, 32), np.float32)
    E_pad[real] = edges[eidx[real]]
    xinit[0:32] = E_pad.reshape(T, 32).T
    xinit[32] = real.reshape(T)

    halo = np.zeros((544, 64), np.float32)
    jj = np.arange(i0 - 32, i0 + 512)
    halo[jj >= 0] = nodes[jj[jj >= 0]]
    nj = (li[:, None] + r[None, :]).reshape(T)
    xinit[33:97] = halo.T[:, nj]

    # weights
    Wan0, ban0 = inputs["Wan0"], inputs["ban0"]
    Wan1, ban1 = inputs["Wan1"], inputs["ban1"]
    Wln0, bln0 = inputs["Wln0"], inputs["bln0"]
    Wln1, bln1 = inputs["Wln1"], inputs["bln1"]
    Wae0, bae0 = inputs["Wae0"], inputs["bae0"]
    Wae1, bae1 = inputs["Wae1"], inputs["bae1"]
    Wle0, ble0 = inputs["Wle0"], inputs["ble0"]
    Wle1, ble1 = inputs["Wle1"], inputs["ble1"]

    tok = np.arange(T)
    m = (tok % 32 != 0).astype(np.float32)
    cnt = (r[None, :] - (32 - tt[:, None])).reshape(T).astype(np.float32)
    nrm = np.where(cnt > 0, 1.0 / np.maximum(cnt, 1), 1.0).astype(np.float32)
    norm_node = np.full(N, 1.0 / M, np.float32)
    norm_node[1:M] = 1.0 / np.arange(1, M)
    nv = norm_node[i0 + np.arange(NL)]

    z31x128 = np.zeros((31, 128), np.float32)
    z31x64 = np.zeros((31, 64), np.float32)
    seg = {
        # K padded to 128 (zero rows against zeroed/ignored X rows) for FWL
        "w_t1a": np.concatenate([np.asarray(Wan0)[64:96], np.asarray(ban0)[None, :],
                                 np.asarray(Wan0)[0:64], z31x128], 0),
        "w_t1b": np.asarray(Wan0)[96:160],
        "w_s1": np.concatenate([np.asarray(Wae0)[64:96], np.asarray(bae0)[None, :],
                                np.asarray(Wae0)[0:64], z31x64], 0),
        "w_t2": np.asarray(Wan1),
        "w_s2d": np.concatenate([np.asarray(Wae1), np.asarray(Wae1)], 0),
        "w_gl0": np.concatenate([np.asarray(Wle0)[32:64], np.asarray(ble0)[None, :],
                                 np.asarray(Wle0)[0:32],
                                 np.zeros((63, 64), np.float32)], 0),
        "w_outed": np.concatenate([np.asarray(Wle1), np.asarray(Wle1)], 0),
        "w_hl0a": np.asarray(Wln0)[0:64],
        "w_hl0b": np.asarray(Wln0)[64:128],
        "w_nl1": np.asarray(Wln1),
        "nodesT": halo.T,
        "normn": np.broadcast_to(nv.reshape(2, 256)[:, None, :],
                                 (2, 64, 256)).reshape(128, 256),
        "t2mask": real.reshape(T)[:1024][None, :].repeat(64, 0),
        "s2mask": real.reshape(T)[:1024][None, :].repeat(32, 0),
    }
    wblob = np.zeros((128, WBLOB_COLS), np.float32)
    for name, arr in seg.items():
        rows, off, cols = _WSEG[name]
        assert arr.shape == (rows, cols), (name, arr.shape)
        wblob[0:rows, off:off + cols] = arr

    tblob = np.zeros((128, TBLOB_COLS), np.float32)
    tblob[:, 0:TG4] = np.broadcast_to(m.reshape(4, TG4)[:, None, :],
                                      (4, 32, TG4)).reshape(128, TG4)
    tblob[:, TG4:] = np.broadcast_to(nrm.reshape(4, TG4)[:, None, :],
                                     (4, 32, TG4)).reshape(128, TG4)

    fblob = np.zeros((128, FBLOB_COLS), np.float32)
    fblob[0:128, 0] = np.concatenate([ban1, ban1])
    fblob[0:128, 1] = np.tile(np.asarray(bae1), 4)
    fblob[0:128, 2] = np.tile(np.asarray(ble1), 4)
    fblob[0:128, 3] = np.asarray(bln0)
    fblob[0:64, 4] = np.asarray(bln1)

    return {"xinit": _bf(xinit), "wblob": _bf(wblob), "tblob": _bf(tblob),
            "fblob": _f32(fblob)}


# --------------------------------------------------------------------------
# device kernel
# --------------------------------------------------------------------------
def build_kernel(debug=False):
    nc = bacc.Bacc("TRN2", target_bir_lowering=False, debug=debug)
    AF = mybir.ActivationFunctionType
    ALU = mybir.AluOpType

    xinit_d = nc.dram_tensor("xinit", [128, T], BF, kind="ExternalInput").ap()
    wblob_d = nc.dram_tensor("wblob", [128, WBLOB_COLS], BF, kind="ExternalInput").ap()
    tblob_d = nc.dram_tensor("tblob", [128, TBLOB_COLS], BF, kind="ExternalInput").ap()
    fblob_d = nc.dram_tensor("fblob", [128, FBLOB_COLS], F32, kind="ExternalInput").ap()
    out_n = nc.dram_tensor("out_n", [64, NL], F32, kind="ExternalOutput").ap()
    out_e = nc.dram_tensor("out_e", [128, TG4], BF, kind="ExternalOutput").ap()

    with tile.TileContext(nc) as tc:
        with (
            tc.tile_pool(name="big", bufs=1) as big,
            tc.tile_pool(name="pt1", bufs=2, space="PSUM") as pt1p,
            tc.tile_pool(name="ppk", bufs=4, space="PSUM") as ppk,
            tc.tile_pool(name="pnd", bufs=2, space="PSUM") as pnd,
        ):
            X = big.tile([128, T], BF, tag="X")
            t1 = big.tile([128, T], BF, tag="t1")
            S = big.tile([128, TG2], BF, tag="S")
            t2 = big.tile([128, TG2], BF, tag="t2")
            s2m = big.tile([128, TG4], BF, tag="s2m")
            incl = big.tile([128, TG4], BF, tag="incl")
            excl = big.tile([128, TG4], BF, tag="excl")
            Gg = big.tile([128, TG2], BF, tag="G")
            oute = big.tile([128, TG4], BF, tag="oute")
            W = big.tile([128, WBLOB_COLS], BF, tag="W")
            Tb = big.tile([128, TBLOB_COLS], BF, tag="Tb")
            Fb = big.tile([128, FBLOB_COLS], F32, tag="Fb")
            prevn_raw = big.tile([128, 256], F32, tag="prevn_raw")
            prevn = big.tile([128, 256], BF, tag="prevn")
            pn2 = big.tile([64, 512], BF, tag="pn2")
            hl0 = big.tile([128, 512], BF, tag="hl0")
            outn_sb = big.tile([64, 512], F32, tag="outn_sb")

            def w(name):
                rows, off, cols = _WSEG[name]
                return W[0:rows, off:off + cols]

            def tb(name):
                rows, off, cols = _TSEG[name]
                return Tb[0:rows, off:off + cols]

            def f(name):
                rows, col = _FSEG[name]
                return Fb[0:rows, col:col + 1]

            dma = nc.sync.dma_start
            dma(out=W[:, :], in_=wblob_d)
            dma(out=Fb[:, :], in_=fblob_d)
            for q in range(4):
                qs = slice(q * TG4, (q + 1) * TG4)
                dma(out=X[:, qs], in_=xinit_d[:, qs])
            dma(out=Tb[:, :], in_=tblob_d)

            def cw(c):
                return slice(c * NCH, (c + 1) * NCH)

            def node_i_bcast(c):
                # rhs [64, 16 blocks, 32 bcast] reading nodesT cols li+32
                base = w("nodesT")[:, 32 + 16 * c: 32 + 16 * c + 16]
                return bass.AP(tensor=base.tensor, offset=base.offset,
                               ap=[base.ap[0], base.ap[1], [0, 32]])

            # ---- PE warmup during input load (HAM un-throttle) ----------
            for _ in range(12):
                pw = pnd.tile([128, 512], F32, tag="pnd")
                nc.tensor.matmul(pw[:, :], w("w_t1a")[:, 0:128],
                                 W[0:128, 0:512], start=True, stop=True)

            # ---- t1 ------------------------------------------------------
            for c in range(NCHUNKS):
                pt = pt1p.tile([128, NCH], F32, tag="pt1")
                nc.tensor.matmul(pt[:, :], w("w_t1a"), X[0:128, cw(c)],
                                 start=True, stop=False)
                nc.tensor.matmul(pt[:, :], w("w_t1b"), node_i_bcast(c),
                                 start=False, stop=True)
                if c % 2 == 0:
                    nc.scalar.activation(t1[:, cw(c)], pt[:, :], AF.Relu)
                else:
                    nc.vector.tensor_scalar_max(t1[:, cw(c)], pt[:, :], 0.0)

            # ---- s1 (pack 2 chunks/psum tile) ---------------------------
            for cp in range(16):
                ps = ppk.tile([128, NCH], F32, tag="ppk")
                nc.tensor.matmul(ps[0:64, :], w("w_s1"), X[0:128, cw(cp)],
                                 start=True, stop=True)
                nc.tensor.matmul(ps[64:128, :], w("w_s1"), X[0:128, cw(cp + 16)],
                                 start=True, stop=True)
                if cp % 2 == 0:
                    nc.scalar.activation(S[:, cw(cp)], ps[:, :], AF.Relu)
                else:
                    nc.vector.tensor_scalar_max(S[:, cw(cp)], ps[:, :], 0.0)

            # ---- s2 (pack 4, ACT bias bae1) -----------------------------
            for cq in range(8):
                ps = ppk.tile([128, NCH], F32, tag="ppk")
                for k in range(4):
                    c = cq + 8 * k
                    g = c // 16
                    col = slice(c * NCH - g * TG2, (c + 1) * NCH - g * TG2)
                    nc.tensor.matmul(ps[32 * k:32 * k + 32, :],
                                     w("w_s2d")[64 * g:64 * g + 64, :],
                                     S[64 * g:64 * g + 64, col],
                                     start=True, stop=True,
                                     tile_position=(64 * g, 32 * k))
                nc.scalar.activation(s2m[:, cw(cq)], ps[:, :], AF.Relu,
                                     bias=f("bae1_t"))
            nc.vector.tensor_mul(s2m[0:32, 0:1024], s2m[0:32, 0:1024],
                                 w("s2mask"))

            # ---- edge prefix scan (column-split into 4; blocks are
            # 32-aligned so each column range scans independently, letting
            # gl0 start as soon as the first range's prev_e lands) ---------
            QW = TG4 // 4
            for qq in (0, 1, 2, 3):
                qs = slice(qq * QW, (qq + 1) * QW)
                nc.vector.tensor_tensor_scan(incl[:, qs], tb("scanmask")[:, qs],
                                             s2m[:, qs], 0.0, ALU.mult, ALU.add)
                nc.vector.tensor_sub(excl[:, qs], incl[:, qs], s2m[:, qs])
                nc.vector.tensor_mul(excl[:, qs], excl[:, qs],
                                     tb("nrm_gs")[:, qs])
                for g in range(4):
                    dma(out=X[33:65, g * TG4 + qq * QW:g * TG4 + (qq + 1) * QW],
                        in_=excl[32 * g:32 * g + 32, qs])

            # ---- t2 (pack 2, ACT bias ban1) — overlaps the scan chain ---
            for cp in range(16):
                ps = ppk.tile([128, NCH], F32, tag="ppk")
                nc.tensor.matmul(ps[0:64, :], w("w_t2"), t1[:, cw(cp)],
                                 start=True, stop=True)
                nc.tensor.matmul(ps[64:128, :], w("w_t2"), t1[:, cw(cp + 16)],
                                 start=True, stop=True)
                nc.scalar.activation(t2[:, cw(cp)], ps[:, :], AF.Relu,
                                     bias=f("ban1_t"))

            # ---- node aggregation (overlaps gl0 on PE) ------------------
            nc.vector.tensor_mul(t2[0:64, 0:1024], t2[0:64, 0:1024],
                                 w("t2mask"))
            t2v = t2[:, :].rearrange("p (b r) -> p b r", r=32)
            nc.vector.reduce_sum(out=prevn_raw[:, :], in_=t2v,
                                 axis=mybir.AxisListType.X)
            nc.vector.tensor_mul(prevn[:, :], prevn_raw[:, :], w("normn"))
            dma(out=pn2[0:64, 0:256], in_=prevn[0:64, :])
            dma(out=pn2[0:64, 256:512], in_=prevn[64:128, :])

            # ---- gl0 (pack 2) -------------------------------------------
            for cp in range(16):
                ps = ppk.tile([128, NCH], F32, tag="ppk")
                nc.tensor.matmul(ps[0:64, :], w("w_gl0"), X[0:128, cw(cp)],
                                 start=True, stop=True)
                nc.tensor.matmul(ps[64:128, :], w("w_gl0"), X[0:128, cw(cp + 16)],
                                 start=True, stop=True)
                if cp % 2 == 0:
                    nc.scalar.activation(Gg[:, cw(cp)], ps[:, :], AF.Relu)
                else:
                    nc.vector.tensor_scalar_max(Gg[:, cw(cp)], ps[:, :], 0.0)

            # ---- node MLP ------------------------------------------------
            ph = pnd.tile([128, 512], F32, tag="pnd")
            nc.tensor.matmul(ph[:, :], w("w_hl0a"), pn2[:, :],
                             start=True, stop=False)
            nc.tensor.matmul(ph[:, :], w("w_hl0b"), w("nodesT")[:, 32:544],
                             start=False, stop=True)
            nc.scalar.activation(hl0[:, :], ph[:, :], AF.Relu, bias=f("bln0_t"))
            po = pnd.tile([128, 512], F32, tag="pnd")
            nc.tensor.matmul(po[0:64, :], w("w_nl1"), hl0[:, :],
                             start=True, stop=True)
            nc.scalar.activation(outn_sb[:, :], po[0:64, :], AF.Relu,
                                 bias=f("bln1_t"))
            dma(out=out_n, in_=outn_sb[:, :])

            # ---- out_e (pack 4, ACT bias ble1) --------------------------
            for cq in range(8):
                ps = ppk.tile([128, NCH], F32, tag="ppk")
                for k in range(4):
                    c = cq + 8 * k
                    g = c // 16
                    col = slice(c * NCH - g * TG2, (c + 1) * NCH - g * TG2)
                    nc.tensor.matmul(ps[32 * k:32 * k + 32, :],
                                     w("w_outed")[64 * g:64 * g + 64, :],
                                     Gg[64 * g:64 * g + 64, col],
                                     start=True, stop=True,
                                     tile_position=(64 * g, 32 * k))
                nc.scalar.activation(oute[:, cw(cq)], ps[:, :], AF.Relu,
                                     bias=f("ble1_t"))
                if cq == 3:
                    dma(out=out_e[:, 0:4 * NCH], in_=oute[:, 0:4 * NCH])
            dma(out=out_e[:, 4 * NCH:], in_=oute[:, 4 * NCH:])

    if not nc.is_finalized():
        nc.finalize()
    return nc


def _install_ntff_shim():
    """Provide antenv.axon_hooks (missing on this image) so trace=True can
    capture NTFF profiles via the axon .so C ABI. Only used when KERNEL_TRACE
    is set; the plain kernel() path never imports it."""
    import contextlib
    import ctypes
    import sys as _sys
    import types

    try:
        from antenv.axon_hooks import get_axon_ntff_profile_hook  # noqa: F401
        return
    except ImportError:
        pass

    so_path = "/opt/axon/libaxon_pjrt.so"
    hook = None
    try:
        lib = ctypes.CDLL(so_path)
        if hasattr(lib, "axon_start_nrt_profile"):
            lib.axon_start_nrt_profile.argtypes = [
                ctypes.POINTER(ctypes.c_int64), ctypes.c_size_t]
            lib.axon_start_nrt_profile.restype = ctypes.c_int64
            lib.axon_stop_nrt_profile.argtypes = [ctypes.c_char_p]
            lib.axon_stop_nrt_profile.restype = ctypes.c_int64

            @contextlib.contextmanager
            def _hook(output_dir, device_ids):
                import jax
                jax.devices()
                if device_ids:
                    ids = (ctypes.c_int64 * len(device_ids))(*device_ids)
                    rc = lib.axon_start_nrt_profile(ids, len(device_ids))
                else:
                    rc = lib.axon_start_nrt_profile(None, 0)
                if rc != 0:
                    raise RuntimeError(f"axon_start_nrt_profile rc={rc}")
                try:
                    yield
                finally:
                    n = lib.axon_stop_nrt_profile(str(output_dir).encode())
                    print(f"ntff profile: {n} file(s) -> {output_dir}")

            hook = _hook
    except OSError:
        pass

    mod = types.ModuleType("antenv.axon_hooks")
    mod._hook = hook
    mod.get_axon_ntff_profile_hook = lambda: mod._hook
    mod.set_axon_ntff_profile_hook = lambda h: setattr(mod, "_hook", h)
    import antenv
    antenv.axon_hooks = mod
    _sys.modules["antenv.axon_hooks"] = mod


# --------------------------------------------------------------------------
# host entry point
# --------------------------------------------------------------------------
def kernel(**inputs):
    in_maps = []
    metas = []
    for core in range(8):
        b, h = core // 2, core % 2
        in_maps.append(_host_prep(b, h, inputs))
        metas.append((b, h))

    nc = build_kernel(debug=False)
    trace = bool(os.environ.get("KERNEL_TRACE"))
    if trace:
        _install_ntff_shim()
    res = run_bass_kernel_spmd(nc, in_maps, core_ids=list(range(8)), trace=trace)
    global LAST_EXEC_NS
    LAST_EXEC_NS = res.exec_time_ns
    results = res.results

    output_nodes = np.zeros((B, N, 64), np.float32)
    output_edges = np.zeros((B, N_E, 32), np.float32)
    for core, (b, h) in enumerate(metas):
        i0 = 512 * h
        real, eidx, _ = _core_tables(i0)
        on = np.asarray(results[core]["out_n"], np.float32)         # [64, 512]
        oe = np.asarray(results[core]["out_e"]).astype(np.float32)  # [128, TG4]
        output_nodes[b, i0:i0 + NL] = on.T
        # unstack: partition p = f + 32*k, col c -> slot = k*TG4 + c
        oe4 = oe.reshape(4, 32, TG4)
        slots = np.transpose(oe4, (0, 2, 1)).reshape(T, 32)
        blk = slots.reshape(NL, 32, 32)
        output_edges[b][eidx[real]] = blk[real]
    return output_nodes, output_edges
